# revision 14
# baseline (speedup 1.0000x reference)
"""CollaborativeAttention Trainium2 kernel.

Full inputs in, full output out. Shards batch (B=8) across 8 NeuronCores,
one batch element per core. All matmuls bf16 with fp32 PSUM accumulation.

Per-core dataflow (batch element b):
  xT [C,N] (host-transposed, bf16) is the moving/stationary operand for all
  input-side matmuls:
    qT[j,n] = sum_c WqT[c,j] * xT[c,n]          (lhsT=WqT tile, rhs=xT)
    kT[j,n] = sum_c WkT[c,j] * xT[c,n]
    v[m,j'] = sum_c xT[c,m] * WvT_aug[c,j']     (lhsT=xT tile, rhs=WvT_aug)
              + bvB_aug (row-replicated bias; ones-cols give the softmax
              denominator for free in the AV matmul)
    cbT[m,h] = sum_c xT[c,m] * WcbT_s[c,h]      (WcbT_s has SCALE folded in)
  per head h:
    khT = kT * mix[h,:] (per-partition scalar on DVE)
    scoresT[m,n] psum = sum_d khT[d,m-cols]^T @ qT[d,n]
    expT[m,n] = exp(SCALE*scoresT + cbT_scaled[m,h])   (ScalarE, bias/scale fused)
    psum_o[dh+1, n] = sum_m v_aug[m, head-block]^T @ expT[m,n]
       row dh (=64) is S[n] = sum_m expT  (from the ones column)
    recip = 1/S ; row-broadcast via K=1 ones-matmul ; ao[dh,h,n] = psum_o * recipB
  output projection (K=64 per head, accumulate 12 heads):
    y[n,c] = sum_h ao[:,h,n-cols]^T @ Wproj64[:,h,c]  + bproj
"""

import numpy as np
import ml_dtypes

B, N, C = 8, 1024, 768
H, Dh = 12, 64
SCALE = Dh ** -0.5
NCORES = 8
BF16 = ml_dtypes.bfloat16

_CACHE = {}


def _chunks(total, size):
    out = []
    off = 0
    while off < total:
        out.append((off, min(size, total - off)))
        off += size
    return out


def emit(ctx, tc, t, C_, N_, H_):
    """Emit the per-core kernel body. t: dict of dram APs."""
    import concourse.mybir as mybir
    from concourse.bass import ts, ds

    nc = tc.nc
    dt = mybir.dt
    CT = C_ // 128          # c/d tiles (contraction over features)
    NT = N_ // 128          # token tiles (n or m)
    JT = C_ // 128          # output-feature tiles for q/k
    VW = H_ * 65            # augmented v width
    NCH = _chunks(N_, 512)  # n chunks for moving operand
    VCH = _chunks(VW, 512)
    CCH = _chunks(C_, 384)  # proj output chunks (<=512, 2 banks-friendly)

    singles = ctx.enter_context(tc.tile_pool(name="singles", bufs=1))
    kh_pool = ctx.enter_context(tc.tile_pool(name="khp", bufs=2))
    exp_pool = ctx.enter_context(tc.tile_pool(name="expp", bufs=2))
    small = ctx.enter_context(tc.tile_pool(name="small", bufs=2))
    ystage = ctx.enter_context(tc.tile_pool(name="ystage", bufs=2))
    psum = ctx.enter_context(tc.tile_pool(name="psum", bufs=6, space="PSUM"))
    psum_o = ctx.enter_context(tc.tile_pool(name="psum_o", bufs=2, space="PSUM"))

    bf = dt.bfloat16
    f32 = dt.float32

    # ---- persistent SBUF tensors ----
    GP = H_ // 2            # head pairs (proj contraction tiles of 128)
    wmixT_s = singles.tile([128, CT, H_], f32, tag="wmixT")
    wproj_s = singles.tile([128, GP, C_], bf, tag="wproj")
    bprojB_s = singles.tile([128, C_], f32, tag="bprojB")

    qT_s = singles.tile([128, JT, N_], bf, tag="qT")
    kT_s = singles.tile([128, JT, N_], bf, tag="kT")
    v_s = singles.tile([128, NT, VW], bf, tag="v")
    cb_s = singles.tile([128, NT, H_], f32, tag="cb")
    ao_s = singles.tile([128, GP, N_], bf, tag="ao")

    nc.sync.dma_start(out=wmixT_s, in_=t["wmixT"].rearrange("(t p) n -> p t n", p=128))
    nc.sync.dma_start(out=wproj_s, in_=t["wproj64"])
    nc.sync.dma_start(out=bprojB_s, in_=t["bprojB"])

    # ---- stage B: projections (inputs scoped to a pool freed afterwards) ----
    with tc.tile_pool(name="stageb", bufs=1) as sbp:
        xT_s = sbp.tile([128, CT, N_], bf, tag="xT")
        wqT_s = sbp.tile([128, CT, C_], bf, tag="wqT")
        wkT_s = sbp.tile([128, CT, C_], bf, tag="wkT")
        wvT_s = sbp.tile([128, CT, VW], bf, tag="wvT")
        wcbT_s = sbp.tile([128, CT, H_], bf, tag="wcbT")
        bvB_s = sbp.tile([128, VW], f32, tag="bvB")

        nc.sync.dma_start(out=xT_s, in_=t["xT"].rearrange("(t p) n -> p t n", p=128))
        nc.sync.dma_start(out=wqT_s, in_=t["wqT"].rearrange("(t p) n -> p t n", p=128))
        nc.sync.dma_start(out=wkT_s, in_=t["wkT"].rearrange("(t p) n -> p t n", p=128))
        nc.sync.dma_start(
            out=wvT_s, in_=t["wvT_aug"].rearrange("(t p) n -> p t n", p=128)
        )
        nc.sync.dma_start(
            out=wcbT_s, in_=t["wcbT_s"].rearrange("(t p) n -> p t n", p=128)
        )
        nc.sync.dma_start(out=bvB_s, in_=t["bvB_aug"])

        # content bias (needed earliest: bias of head-0 exp)
        for mt in range(NT):
            ps = psum.tile([128, 512], f32, tag="ps")
            for ct in range(CT):
                nc.tensor.matmul(
                    ps[:, :H_],
                    lhsT=xT_s[:, ct, ts(mt, 128)],
                    rhs=wcbT_s[:, ct, :],
                    start=(ct == 0),
                    stop=(ct == CT - 1),
                )
            nc.scalar.copy(out=cb_s[:, mt, :], in_=ps[:, :H_])

        # kT then qT (kT needed first for head-0 mix-scale)
        for dst, w_s in ((kT_s, wkT_s), (qT_s, wqT_s)):
            for jt in range(JT):
                for (no, nsz) in NCH:
                    ps = psum.tile([128, 512], f32, tag="ps")
                    for ct in range(CT):
                        nc.tensor.matmul(
                            ps[:, :nsz],
                            lhsT=w_s[:, ct, ts(jt, 128)],
                            rhs=xT_s[:, ct, ds(no, nsz)],
                            start=(ct == 0),
                            stop=(ct == CT - 1),
                        )
                    nc.any.tensor_copy(out=dst[:, jt, ds(no, nsz)], in_=ps[:, :nsz])

        # v (n-major, augmented with ones cols) + bias add
        for mt in range(NT):
            for (vo, vsz) in VCH:
                ps = psum.tile([128, 512], f32, tag="ps")
                for ct in range(CT):
                    nc.tensor.matmul(
                        ps[:, :vsz],
                        lhsT=xT_s[:, ct, ts(mt, 128)],
                        rhs=wvT_s[:, ct, ds(vo, vsz)],
                        start=(ct == 0),
                        stop=(ct == CT - 1),
                    )
                nc.vector.tensor_add(
                    out=v_s[:, mt, ds(vo, vsz)],
                    in0=ps[:, :vsz],
                    in1=bvB_s[:, ds(vo, vsz)],
                )

    # ---- head loop (software-pipelined emission: scores(h) then AV(h-1)) ----
    def emit_scores(h, kh_t, exp_t):
        for dt_i in range(CT):
            nc.vector.tensor_scalar_mul(
                kh_t[:, dt_i, :], kT_s[:, dt_i, :], wmixT_s[:, dt_i, h : h + 1]
            )
        for mt in range(NT):
            for (no, nsz) in NCH:
                ps = psum.tile([128, 512], f32, tag="ps")
                for dt_i in range(CT):
                    nc.tensor.matmul(
                        ps[:, :nsz],
                        lhsT=kh_t[:, dt_i, ts(mt, 128)],
                        rhs=qT_s[:, dt_i, ds(no, nsz)],
                        start=(dt_i == 0),
                        stop=(dt_i == CT - 1),
                    )
                nc.scalar.activation(
                    out=exp_t[:, mt, ds(no, nsz)],
                    in_=ps[:, :nsz],
                    func=mybir.ActivationFunctionType.Exp,
                    bias=cb_s[:, mt, h : h + 1],
                    scale=SCALE,
                )

    def emit_av(h, exp_t):
        for (no, nsz) in NCH:
            po = psum_o.tile([65, 512], f32, tag="po")
            for mt in range(NT):
                nc.tensor.matmul(
                    po[:, :nsz],
                    lhsT=v_s[:, mt, ds(h * 65, 65)],
                    rhs=exp_t[:, mt, ds(no, nsz)],
                    start=(mt == 0),
                    stop=(mt == NT - 1),
                )
            # softmax denominator handling, all off the PE critical path:
            # S row (psum part 64) -> sbuf -> DMA-spread to [128, nsz/128] ->
            # full-width reciprocal -> DMA to DRAM scratch -> broadcast-DMA
            # to 64 partitions -> DVE normalize-multiply.
            b_ = nsz // 128
            nci = no // 512
            s_row = small.tile([65, 512], f32, tag="s_row")
            nc.scalar.copy(out=s_row[64:65, :nsz], in_=po[64:65, :nsz])
            s_sp = small.tile([128, 4], f32, tag="s_sp")
            nc.sync.dma_start(
                out=s_sp[:, :b_],
                in_=s_row[64:65, :nsz].rearrange("p (a b) -> p a b", a=128),
            )
            nc.vector.reciprocal(out=s_sp[:, :b_], in_=s_sp[:, :b_])
            r_dram = t["r_scratch"][h, nci, :, :nsz]  # [1, nsz] DRAM
            nc.sync.dma_start(
                out=r_dram.rearrange("p (a b) -> p a b", a=128), in_=s_sp[:, :b_]
            )
            recipB = small.tile([64, 512], f32, tag="recipB")
            nc.sync.dma_start(
                out=recipB[:, :nsz], in_=r_dram.to_broadcast((64, nsz))
            )
            if h % 2 == 0:
                nc.vector.tensor_mul(
                    out=ao_s[0:64, h // 2, ds(no, nsz)],
                    in0=po[:64, :nsz],
                    in1=recipB[:, :nsz],
                )
            else:
                # odd heads land on partitions 64-127 of the pair tile; DVE
                # can't shift partitions, so normalize into a temp and DMA.
                ao_tmp = small.tile([64, 512], bf, tag="ao_tmp")
                nc.vector.tensor_mul(
                    out=ao_tmp[:, :nsz], in0=po[:64, :nsz], in1=recipB[:, :nsz]
                )
                nc.sync.dma_start(
                    out=ao_s[64:128, h // 2, ds(no, nsz)], in_=ao_tmp[:, :nsz]
                )

    prev = None
    for h in range(H_):
        kh_t = kh_pool.tile([128, CT, N_], bf, tag="kh")
        exp_t = exp_pool.tile([128, NT, N_], bf, tag="exp")
        emit_scores(h, kh_t, exp_t)
        if prev is not None:
            emit_av(prev[0], prev[1])
        prev = (h, exp_t)
    emit_av(prev[0], prev[1])

    # ---- output projection + bproj ----
    for nt in range(NT):
        yst = ystage.tile([128, C_], f32, tag="yst")
        for (co, csz) in CCH:
            ps = psum.tile([128, 512], f32, tag="ps")
            for g in range(GP):
                nc.tensor.matmul(
                    ps[:, :csz],
                    lhsT=ao_s[:, g, ts(nt, 128)],
                    rhs=wproj_s[:, g, ds(co, csz)],
                    start=(g == 0),
                    stop=(g == GP - 1),
                )
            nc.vector.tensor_add(
                out=yst[:, ds(co, csz)], in0=ps[:, :csz], in1=bprojB_s[:, ds(co, csz)]
            )
        nc.sync.dma_start(out=t["y"][ts(nt, 128), :], in_=yst)


def build(C_=C, N_=N, H_=H, ncores=NCORES):
    import concourse.bacc as bacc
    import concourse.mybir as mybir
    import concourse.tile as tile

    dt = mybir.dt
    nc = bacc.Bacc(
        "TRN2", target_bir_lowering=False, debug=False, num_devices=ncores
    )
    VW = H_ * 65
    t = {}
    t["xT"] = nc.dram_tensor("xT", [C_, N_], dt.bfloat16, kind="ExternalInput").ap()
    t["wqT"] = nc.dram_tensor("wqT", [C_, C_], dt.bfloat16, kind="ExternalInput").ap()
    t["wkT"] = nc.dram_tensor("wkT", [C_, C_], dt.bfloat16, kind="ExternalInput").ap()
    t["wvT_aug"] = nc.dram_tensor(
        "wvT_aug", [C_, VW], dt.bfloat16, kind="ExternalInput"
    ).ap()
    t["wcbT_s"] = nc.dram_tensor(
        "wcbT_s", [C_, H_], dt.bfloat16, kind="ExternalInput"
    ).ap()
    t["wmixT"] = nc.dram_tensor(
        "wmixT", [C_, H_], dt.float32, kind="ExternalInput"
    ).ap()
    t["wproj64"] = nc.dram_tensor(
        "wproj64", [128, H_ // 2, C_], dt.bfloat16, kind="ExternalInput"
    ).ap()
    t["bvB_aug"] = nc.dram_tensor(
        "bvB_aug", [128, VW], dt.float32, kind="ExternalInput"
    ).ap()
    t["bprojB"] = nc.dram_tensor(
        "bprojB", [128, C_], dt.float32, kind="ExternalInput"
    ).ap()
    t["y"] = nc.dram_tensor("y", [N_, C_], dt.float32, kind="ExternalOutput").ap()
    t["r_scratch"] = nc.dram_tensor(
        "r_scratch", [H_, (N_ + 511) // 512, 1, 512], dt.float32, kind="Internal"
    ).ap()

    from contextlib import ExitStack

    with tile.TileContext(nc) as tc:
        with ExitStack() as ctx:
            emit(ctx, tc, t, C_, N_, H_)
    nc.compile()
    return nc


def prep_inputs(x, Wq, Wk, Wv, bv, Wmix, Wcb, Wproj, bproj, C_=C, N_=N, H_=H):
    """Host-side: build per-core input maps from full inputs."""
    VW = H_ * 65
    wqT = np.ascontiguousarray(np.asarray(Wq, np.float32).T).astype(BF16)
    wkT = np.ascontiguousarray(np.asarray(Wk, np.float32).T).astype(BF16)
    wvT = np.ascontiguousarray(np.asarray(Wv, np.float32).T)  # [c, j]
    wvT_aug = np.zeros((C_, VW), np.float32)
    bvB_aug = np.zeros((128, VW), np.float32)
    bv = np.asarray(bv, np.float32)
    for h in range(H_):
        wvT_aug[:, 65 * h : 65 * h + 64] = wvT[:, 64 * h : 64 * h + 64]
        bvB_aug[:, 65 * h : 65 * h + 64] = bv[64 * h : 64 * h + 64][None, :]
        bvB_aug[:, 65 * h + 64] = 1.0
    wcbT_s = (np.asarray(Wcb, np.float32).T * SCALE).astype(BF16)
    wmixT = np.ascontiguousarray(np.asarray(Wmix, np.float32).T)
    wprojT = np.asarray(Wproj, np.float32).T  # [j, c]
    wproj64 = np.ascontiguousarray(
        wprojT.reshape(H_ // 2, 128, C_).transpose(1, 0, 2)
    ).astype(BF16)
    bprojB = np.broadcast_to(np.asarray(bproj, np.float32), (128, C_)).copy()

    shared = {
        "wqT": wqT,
        "wkT": wkT,
        "wvT_aug": wvT_aug.astype(BF16),
        "wcbT_s": wcbT_s,
        "wmixT": wmixT,
        "wproj64": wproj64,
        "bvB_aug": bvB_aug,
        "bprojB": bprojB,
    }
    x = np.asarray(x, np.float32)
    in_maps = []
    for b in range(x.shape[0]):
        m = dict(shared)
        m["xT"] = np.ascontiguousarray(x[b].T).astype(BF16)
        in_maps.append(m)
    return in_maps


def kernel(x, Wq, Wk, Wv, bv, Wmix, Wcb, Wproj, bproj):
    from concourse.bass_utils import run_bass_kernel_spmd

    if "nc" not in _CACHE:
        _CACHE["nc"] = build()
    nc = _CACHE["nc"]
    in_maps = prep_inputs(x, Wq, Wk, Wv, bv, Wmix, Wcb, Wproj, bproj)
    res = run_bass_kernel_spmd(nc, in_maps, core_ids=list(range(NCORES)))
    out = np.stack([res.results[b]["y"] for b in range(len(in_maps))], axis=0)
    return out.astype(np.float32)


# revision 39
# speedup vs baseline: 1.8303x; 1.8303x over previous
"""CollaborativeAttention Trainium2 kernel.

Full inputs in, full output out. Shards batch (B=8) across 8 NeuronCores,
one batch element per core (no collectives). Matmuls are bf16 with fp32
PSUM accumulation, except the score path and the q/k input projections,
which run fp8 e4m3 with DoubleRow (2 MACs/cell/cycle); host-side upscales
(MIX_UPSCALE, QK_UPSCALE) keep fp8 operands out of the denormal range and
are divided back out inside the fused exp() scale.

Per-core dataflow (batch element b), everything transposed so the feature
dim lives on partitions and no on-device transposes are ever needed:
  stage B (from host-pretransposed xT [C,N] and weights):
    qT[j,n]  = sum_c WqT[c,j] xT8[c,n]          (fp8 DoubleRow)
    kT[j,n]  = sum_c WkT[c,j] xT8[c,n]          (fp8 DoubleRow, kept bf16)
    v[m,j']  = sum_c xT[c,m] WvT_aug[c,j'] + bvB_aug
               (j' = 12 blocks of [64 v-cols | one ones-col]; the ones
               column makes the AV matmul emit the softmax denominator)
    cbT[m,h] = sum_c xT[c,m] WcbT_s[c,h]        (SCALE prefolded)
  per head h (emission software-pipelined: scores(h) then AV(h-1)):
    khT = kT * mix[h,:]   (DVE per-partition scalar, fp8 out)
    scoresT[m,n] psum = sum_d khT[d,m]^T qT8[d,n]     (fp8 DoubleRow)
    expT[m,n] = exp(scale*scoresT + cbT[m,h])         (ScalarE, fused)
    psum_o[65,n] = sum_m v_aug[m, block_h]^T expT[m,n]; row 64 = S[n]
    normalize off the PE path: S row -> DRAM -> broadcast-DMA to 64
    partitions -> reciprocal_approx_fast -> DVE multiply into ao.
    Odd heads DMA-shift to partitions 64-127 so ao packs head PAIRS
    on 128 partitions (K=128 output projection with FWL).
  output projection, split so pairs 0..4 fill the kernel-tail PE gap
  while the last head finishes; pair 5 accumulates via SBUF (+bproj).
"""

import numpy as np
import ml_dtypes

B, N, C = 8, 1024, 768
H, Dh = 12, 64
SCALE = Dh ** -0.5
NCORES = 8
BF16 = ml_dtypes.bfloat16

# fp8 (e4m3 + DoubleRow) for the score matmuls; k*mix is pre-scaled by
# MIX_UPSCALE on the host so values clear the e4m3 denormal floor, and the
# exp() scale divides it back out.
FP8_SCORES = True
MIX_UPSCALE = 32.0
# fp8 DoubleRow for the q/k input projections; Wq/Wk are upscaled by
# QK_UPSCALE on the host (their ~0.02-scale values are denormal in e4m3),
# and the exp() scale divides the product back out.
FP8_QKPROJ = True
QK_UPSCALE = 32.0

_CACHE = {}


def _chunks(total, size):
    out = []
    off = 0
    while off < total:
        out.append((off, min(size, total - off)))
        off += size
    return out


def emit(ctx, tc, t, C_, N_, H_):
    """Emit the per-core kernel body. t: dict of dram APs."""
    import concourse.mybir as mybir
    from concourse.bass import ts, ds

    nc = tc.nc
    dt = mybir.dt
    CT = C_ // 128          # c/d tiles (contraction over features)
    NT = N_ // 128          # token tiles (n or m)
    JT = C_ // 128          # output-feature tiles for q/k
    VW = H_ * 65            # augmented v width
    NCH = _chunks(N_, 512)  # n chunks for moving operand
    VCH = _chunks(VW, 512)
    CCH = _chunks(C_, 384)  # proj output chunks (<=512, 2 banks-friendly)

    singles = ctx.enter_context(tc.tile_pool(name="singles", bufs=1))
    kh_pool = ctx.enter_context(tc.tile_pool(name="khp", bufs=2))
    exp_pool = ctx.enter_context(tc.tile_pool(name="expp", bufs=2))
    small = ctx.enter_context(tc.tile_pool(name="small", bufs=2))
    ystage = ctx.enter_context(tc.tile_pool(name="ystage", bufs=2))
    psum = ctx.enter_context(tc.tile_pool(name="psum", bufs=3, space="PSUM"))
    psum_o = ctx.enter_context(tc.tile_pool(name="psum_o", bufs=2, space="PSUM"))
    psum_y = ctx.enter_context(tc.tile_pool(name="psum_y", bufs=3, space="PSUM"))

    bf = dt.bfloat16
    f32 = dt.float32
    f8 = dt.float8e4
    qdt = f8 if FP8_SCORES else bf
    exp_scale = SCALE / MIX_UPSCALE if FP8_SCORES else SCALE
    if FP8_QKPROJ:
        exp_scale = exp_scale / (QK_UPSCALE * QK_UPSCALE)

    # ---- persistent SBUF tensors ----
    GP = H_ // 2            # head pairs (proj contraction tiles of 128)
    wmixT_s = singles.tile([128, CT, H_], f32, tag="wmixT")
    wproj_s = singles.tile([128, GP, C_], bf, tag="wproj")
    bprojB_s = singles.tile([128, C_], f32, tag="bprojB")

    qT_s = singles.tile([128, JT, N_], qdt, tag="qT")
    kT_s = singles.tile([128, JT, N_], bf, tag="kT")
    v_s = singles.tile([128, NT, VW], bf, tag="v")
    cb_s = singles.tile([128, NT, H_], f32, tag="cb")
    ao_s = singles.tile([128, GP, N_], bf, tag="ao")

    # ---- stage B: projections (inputs scoped to a pool freed afterwards) ----
    with tc.tile_pool(name="stageb", bufs=1) as sbp:
        xT_s = sbp.tile([128, CT, N_], bf, tag="xT")
        qk_dt = f8 if FP8_QKPROJ else bf
        if FP8_QKPROJ:
            xT8_s = sbp.tile([128, CT, N_], qk_dt, tag="xT8")
        else:
            xT8_s = xT_s
        wqT_s = sbp.tile([128, CT, C_], qk_dt, tag="wqT")
        wkT_s = sbp.tile([128, CT, C_], qk_dt, tag="wkT")
        wvT_s = sbp.tile([128, CT, VW], bf, tag="wvT")
        wcbT_s = sbp.tile([128, CT, H_], bf, tag="wcbT")
        bvB_s = sbp.tile([128, VW], f32, tag="bvB")

        # per-c-tile DMAs, compute-first order, so matmul accumulation can
        # begin as soon as the first tiles land
        xT_d = t["xT"].rearrange("(t p) n -> p t n", p=128)
        wq_d = t["wqT"].rearrange("(t p) n -> p t n", p=128)
        wk_d = t["wkT"].rearrange("(t p) n -> p t n", p=128)
        wv_d = t["wvT_aug"].rearrange("(t p) n -> p t n", p=128)
        nc.sync.dma_start(out=xT_s[:, 0, :], in_=xT_d[:, 0, :])
        nc.scalar.dma_start(
            out=wcbT_s, in_=t["wcbT_s"].rearrange("(t p) n -> p t n", p=128)
        )
        if FP8_QKPROJ:
            xT8_d = t["xT8"].rearrange("(t p) n -> p t n", p=128)
            for ct in range(CT):
                nc.scalar.dma_start(out=xT8_s[:, ct, :], in_=xT8_d[:, ct, :])
        for ct in range(1, CT):
            nc.sync.dma_start(out=xT_s[:, ct, :], in_=xT_d[:, ct, :])
        for ct in range(CT):
            nc.scalar.dma_start(out=wkT_s[:, ct, :], in_=wk_d[:, ct, :])
        for ct in range(CT):
            nc.sync.dma_start(out=wqT_s[:, ct, :], in_=wq_d[:, ct, :])
        for ct in range(CT):
            nc.sync.dma_start(out=wvT_s[:, ct, :], in_=wv_d[:, ct, :])
        nc.sync.dma_start(out=bvB_s, in_=t["bvB_aug"])
        nc.scalar.dma_start(
            out=wmixT_s, in_=t["wmixT"].rearrange("(t p) n -> p t n", p=128)
        )
        nc.sync.dma_start(out=wproj_s, in_=t["wproj64"])
        nc.sync.dma_start(out=bprojB_s, in_=t["bprojB"])

        # content bias (needed earliest: bias of head-0 exp)
        for mt in range(NT):
            ps = psum.tile([128, 512], f32, tag="ps")
            for ct in range(CT):
                nc.tensor.matmul(
                    ps[:, :H_],
                    lhsT=xT_s[:, ct, ts(mt, 128)],
                    rhs=wcbT_s[:, ct, :],
                    start=(ct == 0),
                    stop=(ct == CT - 1),
                )
            nc.scalar.copy(out=cb_s[:, mt, :], in_=ps[:, :H_])

        # kT then qT (kT needed first for head-0 mix-scale)
        for dst, w_s in ((kT_s, wkT_s), (qT_s, wqT_s)):
            for jt in range(JT):
                for (no, nsz) in NCH:
                    ps = psum.tile([128, 512], f32, tag="ps")
                    if FP8_QKPROJ:
                        for ct in range(0, CT, 2):
                            nc.tensor.matmul(
                                ps[:, :nsz],
                                lhsT=w_s[:, ct : ct + 2, ts(jt, 128)],
                                rhs=xT8_s[:, ct : ct + 2, ds(no, nsz)],
                                start=(ct == 0),
                                stop=(ct == CT - 2),
                                perf_mode=mybir.MatmulPerfMode.DoubleRow,
                            )
                    else:
                        for ct in range(CT):
                            nc.tensor.matmul(
                                ps[:, :nsz],
                                lhsT=w_s[:, ct, ts(jt, 128)],
                                rhs=xT_s[:, ct, ds(no, nsz)],
                                start=(ct == 0),
                                stop=(ct == CT - 1),
                            )
                    nc.any.tensor_copy(out=dst[:, jt, ds(no, nsz)], in_=ps[:, :nsz])

        # v (n-major, augmented with ones cols) + bias add
        for mt in range(NT):
            for (vo, vsz) in VCH:
                ps = psum.tile([128, 512], f32, tag="ps")
                for ct in range(CT):
                    nc.tensor.matmul(
                        ps[:, :vsz],
                        lhsT=xT_s[:, ct, ts(mt, 128)],
                        rhs=wvT_s[:, ct, ds(vo, vsz)],
                        start=(ct == 0),
                        stop=(ct == CT - 1),
                    )
                nc.vector.tensor_add(
                    out=v_s[:, mt, ds(vo, vsz)],
                    in0=ps[:, :vsz],
                    in1=bvB_s[:, ds(vo, vsz)],
                )

    # ---- head loop (software-pipelined emission: scores(h) then AV(h-1)) ----
    def emit_scores(h, kh_t, exp_t):
        for dt_i in range(CT):
            nc.vector.tensor_scalar_mul(
                kh_t[:, dt_i, :], kT_s[:, dt_i, :], wmixT_s[:, dt_i, h : h + 1]
            )
        for mt in range(NT):
            for (no, nsz) in NCH:
                ps = psum.tile([128, 512], f32, tag="ps")
                if FP8_SCORES:
                    for di in range(0, CT, 2):
                        nc.tensor.matmul(
                            ps[:, :nsz],
                            lhsT=kh_t[:, di : di + 2, ts(mt, 128)],
                            rhs=qT_s[:, di : di + 2, ds(no, nsz)],
                            start=(di == 0),
                            stop=(di == CT - 2),
                            perf_mode=mybir.MatmulPerfMode.DoubleRow,
                        )
                else:
                    for di in range(CT):
                        nc.tensor.matmul(
                            ps[:, :nsz],
                            lhsT=kh_t[:, di, ts(mt, 128)],
                            rhs=qT_s[:, di, ds(no, nsz)],
                            start=(di == 0),
                            stop=(di == CT - 1),
                        )
                nc.scalar.activation(
                    out=exp_t[:, mt, ds(no, nsz)],
                    in_=ps[:, :nsz],
                    func=mybir.ActivationFunctionType.Exp,
                    bias=cb_s[:, mt, h : h + 1],
                    scale=exp_scale,
                )

    def emit_av(h, exp_t, fast_tail=False):
        for (no, nsz) in NCH:
            po = psum_o.tile([65, 512], f32, tag="po")
            for mt in range(NT):
                nc.tensor.matmul(
                    po[:, :nsz],
                    lhsT=v_s[:, mt, ds(h * 65, 65)],
                    rhs=exp_t[:, mt, ds(no, nsz)],
                    start=(mt == 0),
                    stop=(mt == NT - 1),
                )
            # softmax denominator, off the PE critical path: one broadcast-DMA
            # of the S row (psum part 64) to 64 partitions, then a full-width
            # fast reciprocal and the normalize multiply on DVE.
            nci = no // 512
            s_row = small.tile([65, 512], f32, tag="s_row")
            nc.scalar.copy(out=s_row[64:65, :nsz], in_=po[64:65, :nsz])
            recipB = small.tile([64, 512], f32, tag="recipB")
            r_dram = t["r_scratch"][h, nci, :, :nsz]  # [1, nsz] DRAM
            nc.sync.dma_start(out=r_dram, in_=s_row[64:65, :nsz])
            nc.sync.dma_start(
                out=recipB[:, :nsz], in_=r_dram.to_broadcast((64, nsz))
            )
            nc.vector.reciprocal_approx_fast(
                out=recipB[:, :nsz], in_=recipB[:, :nsz]
            )
            if h % 2 == 0:
                nc.vector.tensor_mul(
                    out=ao_s[0:64, h // 2, ds(no, nsz)],
                    in0=po[:64, :nsz],
                    in1=recipB[:, :nsz],
                )
            else:
                # odd heads land on partitions 64-127 of the pair tile; DVE
                # can't shift partitions, so normalize into a temp and DMA.
                ao_tmp = small.tile([64, 512], bf, tag="ao_tmp")
                nc.vector.tensor_mul(
                    out=ao_tmp[:, :nsz], in0=po[:64, :nsz], in1=recipB[:, :nsz]
                )
                nc.sync.dma_start(
                    out=ao_s[64:128, h // 2, ds(no, nsz)], in_=ao_tmp[:, :nsz]
                )

    prev = None
    head_order = list(range(H_))
    if H_ >= 2:
        head_order[-2], head_order[-1] = head_order[-1], head_order[-2]
    for h in head_order:
        kh_t = kh_pool.tile([128, CT, N_], qdt, tag="kh")
        exp_t = exp_pool.tile([128, NT, N_], bf, tag="exp")
        emit_scores(h, kh_t, exp_t)
        if prev is not None:
            emit_av(prev[0], prev[1])
        prev = (h, exp_t)

    # ---- output projection + bproj ----
    # Pairs 0..GP-2 (heads 0..H-3) are final once av(H-3)'s normalize lands,
    # so their proj matmuls are emitted BEFORE the last head's AV to fill the
    # kernel-tail PE gap; the last pair accumulates on top from SBUF.
    if GP > 1:
        yacc_s = singles.tile([128, NT, C_], f32, tag="yacc")
        for nt in range(NT):
            for (co, csz) in CCH:
                ps = psum_y.tile([128, 512], f32, tag="psy")
                for g in range(GP - 1):
                    nc.tensor.matmul(
                        ps[:, :csz],
                        lhsT=ao_s[:, g, ts(nt, 128)],
                        rhs=wproj_s[:, g, ds(co, csz)],
                        start=(g == 0),
                        stop=(g == GP - 2),
                    )
                nc.vector.tensor_add(
                    out=yacc_s[:, nt, ds(co, csz)],
                    in0=ps[:, :csz],
                    in1=bprojB_s[:, ds(co, csz)],
                )

    emit_av(prev[0], prev[1], fast_tail=True)

    for nt in range(NT):
        yst = ystage.tile([128, C_], f32, tag="yst")
        for (co, csz) in CCH:
            # alternate pools: scores' pool is free by now, doubling the
            # banks in flight so the DVE adds never stall the matmuls
            if (nt * len(CCH) + (co // 384)) % 2 == 0:
                ps = psum_y.tile([128, 512], f32, tag="psy")
            else:
                ps = psum.tile([128, 512], f32, tag="ps")
            nc.tensor.matmul(
                ps[:, :csz],
                lhsT=ao_s[:, GP - 1, ts(nt, 128)],
                rhs=wproj_s[:, GP - 1, ds(co, csz)],
                start=True,
                stop=True,
            )
            if GP > 1:
                nc.vector.tensor_add(
                    out=yst[:, ds(co, csz)],
                    in0=ps[:, :csz],
                    in1=yacc_s[:, nt, ds(co, csz)],
                )
            else:
                nc.vector.tensor_add(
                    out=yst[:, ds(co, csz)],
                    in0=ps[:, :csz],
                    in1=bprojB_s[:, ds(co, csz)],
                )
        nc.sync.dma_start(out=t["y"][ts(nt, 128), :], in_=yst)


def build(C_=C, N_=N, H_=H, ncores=NCORES):
    import concourse.bacc as bacc
    import concourse.mybir as mybir
    import concourse.tile as tile

    dt = mybir.dt
    nc = bacc.Bacc(
        "TRN2", target_bir_lowering=False, debug=False, num_devices=ncores
    )
    VW = H_ * 65
    t = {}
    t["xT"] = nc.dram_tensor("xT", [C_, N_], dt.bfloat16, kind="ExternalInput").ap()
    qk_dt = dt.float8e4 if FP8_QKPROJ else dt.bfloat16
    if FP8_QKPROJ:
        t["xT8"] = nc.dram_tensor(
            "xT8", [C_, N_], dt.float8e4, kind="ExternalInput"
        ).ap()
    t["wqT"] = nc.dram_tensor("wqT", [C_, C_], qk_dt, kind="ExternalInput").ap()
    t["wkT"] = nc.dram_tensor("wkT", [C_, C_], qk_dt, kind="ExternalInput").ap()
    t["wvT_aug"] = nc.dram_tensor(
        "wvT_aug", [C_, VW], dt.bfloat16, kind="ExternalInput"
    ).ap()
    t["wcbT_s"] = nc.dram_tensor(
        "wcbT_s", [C_, H_], dt.bfloat16, kind="ExternalInput"
    ).ap()
    t["wmixT"] = nc.dram_tensor(
        "wmixT", [C_, H_], dt.float32, kind="ExternalInput"
    ).ap()
    t["wproj64"] = nc.dram_tensor(
        "wproj64", [128, H_ // 2, C_], dt.bfloat16, kind="ExternalInput"
    ).ap()
    t["bvB_aug"] = nc.dram_tensor(
        "bvB_aug", [128, VW], dt.float32, kind="ExternalInput"
    ).ap()
    t["bprojB"] = nc.dram_tensor(
        "bprojB", [128, C_], dt.float32, kind="ExternalInput"
    ).ap()
    t["y"] = nc.dram_tensor("y", [N_, C_], dt.float32, kind="ExternalOutput").ap()
    t["r_scratch"] = nc.dram_tensor(
        "r_scratch", [H_, (N_ + 511) // 512, 1, 512], dt.float32, kind="Internal"
    ).ap()

    from contextlib import ExitStack

    with tile.TileContext(nc) as tc:
        with ExitStack() as ctx:
            emit(ctx, tc, t, C_, N_, H_)
    nc.compile()
    return nc


def prep_inputs(x, Wq, Wk, Wv, bv, Wmix, Wcb, Wproj, bproj, C_=C, N_=N, H_=H):
    """Host-side: build per-core input maps from full inputs."""
    VW = H_ * 65
    import ml_dtypes as _md
    F8 = _md.float8_e4m3
    if FP8_QKPROJ:
        wqT = np.ascontiguousarray(np.asarray(Wq, np.float32).T * QK_UPSCALE).astype(F8)
        wkT = np.ascontiguousarray(np.asarray(Wk, np.float32).T * QK_UPSCALE).astype(F8)
    else:
        wqT = np.ascontiguousarray(np.asarray(Wq, np.float32).T).astype(BF16)
        wkT = np.ascontiguousarray(np.asarray(Wk, np.float32).T).astype(BF16)
    wvT = np.ascontiguousarray(np.asarray(Wv, np.float32).T)  # [c, j]
    wvT_aug = np.zeros((C_, VW), np.float32)
    bvB_aug = np.zeros((128, VW), np.float32)
    bv = np.asarray(bv, np.float32)
    for h in range(H_):
        wvT_aug[:, 65 * h : 65 * h + 64] = wvT[:, 64 * h : 64 * h + 64]
        bvB_aug[:, 65 * h : 65 * h + 64] = bv[64 * h : 64 * h + 64][None, :]
        bvB_aug[:, 65 * h + 64] = 1.0
    wcbT_s = (np.asarray(Wcb, np.float32).T * SCALE).astype(BF16)
    wmixT = np.ascontiguousarray(np.asarray(Wmix, np.float32).T)
    if FP8_SCORES:
        wmixT = wmixT * MIX_UPSCALE
    wprojT = np.asarray(Wproj, np.float32).T  # [j, c]
    wproj64 = np.ascontiguousarray(
        wprojT.reshape(H_ // 2, 128, C_).transpose(1, 0, 2)
    ).astype(BF16)
    bprojB = np.broadcast_to(np.asarray(bproj, np.float32), (128, C_)).copy()

    shared = {
        "wqT": wqT,
        "wkT": wkT,
        "wvT_aug": wvT_aug.astype(BF16),
        "wcbT_s": wcbT_s,
        "wmixT": wmixT,
        "wproj64": wproj64,
        "bvB_aug": bvB_aug,
        "bprojB": bprojB,
    }
    x = np.asarray(x, np.float32)
    in_maps = []
    for b in range(x.shape[0]):
        m = dict(shared)
        xb = np.ascontiguousarray(x[b].T)
        m["xT"] = xb.astype(BF16)
        if FP8_QKPROJ:
            m["xT8"] = xb.astype(F8)
        in_maps.append(m)
    return in_maps


def kernel(x, Wq, Wk, Wv, bv, Wmix, Wcb, Wproj, bproj):
    from concourse.bass_utils import run_bass_kernel_spmd

    if "nc" not in _CACHE:
        _CACHE["nc"] = build()
    nc = _CACHE["nc"]
    in_maps = prep_inputs(x, Wq, Wk, Wv, bv, Wmix, Wcb, Wproj, bproj)
    res = run_bass_kernel_spmd(nc, in_maps, core_ids=list(range(NCORES)))
    out = np.stack([res.results[b]["y"] for b in range(len(in_maps))], axis=0)
    return out.astype(np.float32)


# revision 40
# speedup vs baseline: 1.8888x; 1.0320x over previous
"""CollaborativeAttention Trainium2 kernel.

Full inputs in, full output out. Shards batch (B=8) across 8 NeuronCores,
one batch element per core (no collectives). Matmuls are bf16 with fp32
PSUM accumulation, except the score path and the q/k input projections,
which run fp8 e4m3 with DoubleRow (2 MACs/cell/cycle); host-side upscales
(MIX_UPSCALE, QK_UPSCALE) keep fp8 operands out of the denormal range and
are divided back out inside the fused exp() scale.

Per-core dataflow (batch element b), everything transposed so the feature
dim lives on partitions and no on-device transposes are ever needed:
  stage B (from host-pretransposed xT [C,N] and weights):
    qT[j,n]  = sum_c WqT[c,j] xT8[c,n]          (fp8 DoubleRow)
    kT[j,n]  = sum_c WkT[c,j] xT8[c,n]          (fp8 DoubleRow, kept bf16)
    v[m,j']  = sum_c xT[c,m] WvT_aug[c,j'] + bvB_aug
               (j' = 12 blocks of [64 v-cols | one ones-col]; the ones
               column makes the AV matmul emit the softmax denominator)
    cbT[m,h] = sum_c xT[c,m] WcbT_s[c,h]        (SCALE prefolded)
  per head h (emission software-pipelined: scores(h) then AV(h-1)):
    khT = kT * mix[h,:]   (DVE per-partition scalar, fp8 out)
    scoresT[m,n] psum = sum_d khT[d,m]^T qT8[d,n]     (fp8 DoubleRow)
    expT[m,n] = exp(scale*scoresT + cbT[m,h])         (ScalarE, fused)
    psum_o[65,n] = sum_m v_aug[m, block_h]^T expT[m,n]; row 64 = S[n]
    normalize off the PE path: S row -> DRAM -> broadcast-DMA to 64
    partitions -> reciprocal_approx_fast -> DVE multiply into ao.
    Odd heads DMA-shift to partitions 64-127 so ao packs head PAIRS
    on 128 partitions (K=128 output projection with FWL).
  output projection, split so pairs 0..4 fill the kernel-tail PE gap
  while the last head finishes; pair 5 accumulates via SBUF (+bproj).
"""

import numpy as np
import ml_dtypes

B, N, C = 8, 1024, 768
H, Dh = 12, 64
SCALE = Dh ** -0.5
NCORES = 8
BF16 = ml_dtypes.bfloat16

# fp8 (e4m3 + DoubleRow) for the score matmuls; k*mix is pre-scaled by
# MIX_UPSCALE on the host so values clear the e4m3 denormal floor, and the
# exp() scale divides it back out.
FP8_SCORES = True
MIX_UPSCALE = 32.0
# fp8 DoubleRow for the q/k input projections; Wq/Wk are upscaled by
# QK_UPSCALE on the host (their ~0.02-scale values are denormal in e4m3),
# and the exp() scale divides the product back out.
FP8_QKPROJ = True
QK_UPSCALE = 32.0

_CACHE = {}


def _chunks(total, size):
    out = []
    off = 0
    while off < total:
        out.append((off, min(size, total - off)))
        off += size
    return out


def emit(ctx, tc, t, C_, N_, H_):
    """Emit the per-core kernel body. t: dict of dram APs."""
    import concourse.mybir as mybir
    from concourse.bass import ts, ds

    nc = tc.nc
    dt = mybir.dt
    CT = C_ // 128          # c/d tiles (contraction over features)
    NT = N_ // 128          # token tiles (n or m)
    JT = C_ // 128          # output-feature tiles for q/k
    VW = H_ * 65            # augmented v width
    NCH = _chunks(N_, 512)  # n chunks for moving operand
    VCH = _chunks(VW, 512)
    CCH = _chunks(C_, 384)  # proj output chunks (<=512, 2 banks-friendly)

    singles = ctx.enter_context(tc.tile_pool(name="singles", bufs=1))
    kh_pool = ctx.enter_context(tc.tile_pool(name="khp", bufs=2))
    exp_pool = ctx.enter_context(tc.tile_pool(name="expp", bufs=2))
    small = ctx.enter_context(tc.tile_pool(name="small", bufs=4))
    ystage = ctx.enter_context(tc.tile_pool(name="ystage", bufs=2))
    psum = ctx.enter_context(tc.tile_pool(name="psum", bufs=3, space="PSUM"))
    psum_o = ctx.enter_context(tc.tile_pool(name="psum_o", bufs=3, space="PSUM"))
    psum_y = ctx.enter_context(tc.tile_pool(name="psum_y", bufs=2, space="PSUM"))

    bf = dt.bfloat16
    f32 = dt.float32
    f8 = dt.float8e4
    qdt = f8 if FP8_SCORES else bf
    exp_scale = SCALE / MIX_UPSCALE if FP8_SCORES else SCALE
    if FP8_QKPROJ:
        exp_scale = exp_scale / (QK_UPSCALE * QK_UPSCALE)

    # ---- persistent SBUF tensors ----
    GP = H_ // 2            # head pairs (proj contraction tiles of 128)
    wmixT_s = singles.tile([128, CT, H_], f32, tag="wmixT")
    wproj_s = singles.tile([128, GP, C_], bf, tag="wproj")
    bprojB_s = singles.tile([128, C_], f32, tag="bprojB")

    qT_s = singles.tile([128, JT, N_], qdt, tag="qT")
    kT_s = singles.tile([128, JT, N_], bf, tag="kT")
    v_s = singles.tile([128, NT, VW], bf, tag="v")
    cb_s = singles.tile([128, NT, H_], f32, tag="cb")
    ao_s = singles.tile([128, GP, N_], bf, tag="ao")

    # ---- stage B: projections (inputs scoped to a pool freed afterwards) ----
    with tc.tile_pool(name="stageb", bufs=1) as sbp:
        xT_s = sbp.tile([128, CT, N_], bf, tag="xT")
        qk_dt = f8 if FP8_QKPROJ else bf
        if FP8_QKPROJ:
            xT8_s = sbp.tile([128, CT, N_], qk_dt, tag="xT8")
        else:
            xT8_s = xT_s
        wqT_s = sbp.tile([128, CT, C_], qk_dt, tag="wqT")
        wkT_s = sbp.tile([128, CT, C_], qk_dt, tag="wkT")
        wvT_s = sbp.tile([128, CT, VW], bf, tag="wvT")
        wcbT_s = sbp.tile([128, CT, H_], bf, tag="wcbT")
        bvB_s = sbp.tile([128, VW], f32, tag="bvB")

        # per-c-tile DMAs, compute-first order, so matmul accumulation can
        # begin as soon as the first tiles land
        xT_d = t["xT"].rearrange("(t p) n -> p t n", p=128)
        wq_d = t["wqT"].rearrange("(t p) n -> p t n", p=128)
        wk_d = t["wkT"].rearrange("(t p) n -> p t n", p=128)
        wv_d = t["wvT_aug"].rearrange("(t p) n -> p t n", p=128)
        nc.sync.dma_start(out=xT_s[:, 0, :], in_=xT_d[:, 0, :])
        nc.scalar.dma_start(
            out=wcbT_s, in_=t["wcbT_s"].rearrange("(t p) n -> p t n", p=128)
        )
        if FP8_QKPROJ:
            xT8_d = t["xT8"].rearrange("(t p) n -> p t n", p=128)
            for ct in range(CT):
                nc.scalar.dma_start(out=xT8_s[:, ct, :], in_=xT8_d[:, ct, :])
        for ct in range(1, CT):
            nc.sync.dma_start(out=xT_s[:, ct, :], in_=xT_d[:, ct, :])
        for ct in range(CT):
            nc.scalar.dma_start(out=wkT_s[:, ct, :], in_=wk_d[:, ct, :])
        for ct in range(CT):
            nc.sync.dma_start(out=wqT_s[:, ct, :], in_=wq_d[:, ct, :])
        for ct in range(CT):
            nc.sync.dma_start(out=wvT_s[:, ct, :], in_=wv_d[:, ct, :])
        nc.sync.dma_start(out=bvB_s, in_=t["bvB_aug"])
        nc.scalar.dma_start(
            out=wmixT_s, in_=t["wmixT"].rearrange("(t p) n -> p t n", p=128)
        )
        nc.sync.dma_start(out=wproj_s, in_=t["wproj64"])
        nc.sync.dma_start(out=bprojB_s, in_=t["bprojB"])

        # content bias (needed earliest: bias of head-0 exp)
        for mt in range(NT):
            ps = psum.tile([128, 512], f32, tag="ps")
            for ct in range(CT):
                nc.tensor.matmul(
                    ps[:, :H_],
                    lhsT=xT_s[:, ct, ts(mt, 128)],
                    rhs=wcbT_s[:, ct, :],
                    start=(ct == 0),
                    stop=(ct == CT - 1),
                )
            nc.scalar.copy(out=cb_s[:, mt, :], in_=ps[:, :H_])

        # kT then qT (kT needed first for head-0 mix-scale)
        for dst, w_s in ((kT_s, wkT_s), (qT_s, wqT_s)):
            for jt in range(JT):
                for (no, nsz) in NCH:
                    ps = psum.tile([128, 512], f32, tag="ps")
                    if FP8_QKPROJ:
                        for ct in range(0, CT, 2):
                            nc.tensor.matmul(
                                ps[:, :nsz],
                                lhsT=w_s[:, ct : ct + 2, ts(jt, 128)],
                                rhs=xT8_s[:, ct : ct + 2, ds(no, nsz)],
                                start=(ct == 0),
                                stop=(ct == CT - 2),
                                perf_mode=mybir.MatmulPerfMode.DoubleRow,
                            )
                    else:
                        for ct in range(CT):
                            nc.tensor.matmul(
                                ps[:, :nsz],
                                lhsT=w_s[:, ct, ts(jt, 128)],
                                rhs=xT_s[:, ct, ds(no, nsz)],
                                start=(ct == 0),
                                stop=(ct == CT - 1),
                            )
                    nc.any.tensor_copy(out=dst[:, jt, ds(no, nsz)], in_=ps[:, :nsz])

        # v (n-major, augmented with ones cols) + bias add
        for mt in range(NT):
            for (vo, vsz) in VCH:
                ps = psum.tile([128, 512], f32, tag="ps")
                for ct in range(CT):
                    nc.tensor.matmul(
                        ps[:, :vsz],
                        lhsT=xT_s[:, ct, ts(mt, 128)],
                        rhs=wvT_s[:, ct, ds(vo, vsz)],
                        start=(ct == 0),
                        stop=(ct == CT - 1),
                    )
                nc.vector.tensor_add(
                    out=v_s[:, mt, ds(vo, vsz)],
                    in0=ps[:, :vsz],
                    in1=bvB_s[:, ds(vo, vsz)],
                )

    # ---- head loop (software-pipelined emission: scores(h) then AV(h-1)) ----
    def emit_scores(h, kh_t, exp_t):
        for dt_i in range(CT):
            nc.vector.tensor_scalar_mul(
                kh_t[:, dt_i, :], kT_s[:, dt_i, :], wmixT_s[:, dt_i, h : h + 1]
            )
        for mt in range(NT):
            for (no, nsz) in NCH:
                ps = psum.tile([128, 512], f32, tag="ps")
                if FP8_SCORES:
                    for di in range(0, CT, 2):
                        nc.tensor.matmul(
                            ps[:, :nsz],
                            lhsT=kh_t[:, di : di + 2, ts(mt, 128)],
                            rhs=qT_s[:, di : di + 2, ds(no, nsz)],
                            start=(di == 0),
                            stop=(di == CT - 2),
                            perf_mode=mybir.MatmulPerfMode.DoubleRow,
                        )
                else:
                    for di in range(CT):
                        nc.tensor.matmul(
                            ps[:, :nsz],
                            lhsT=kh_t[:, di, ts(mt, 128)],
                            rhs=qT_s[:, di, ds(no, nsz)],
                            start=(di == 0),
                            stop=(di == CT - 1),
                        )
                nc.scalar.activation(
                    out=exp_t[:, mt, ds(no, nsz)],
                    in_=ps[:, :nsz],
                    func=mybir.ActivationFunctionType.Exp,
                    bias=cb_s[:, mt, h : h + 1],
                    scale=exp_scale,
                )

    def emit_av(h, exp_t, fast_tail=False):
        for (no, nsz) in NCH:
            po = psum_o.tile([65, 512], f32, tag="po")
            for mt in range(NT):
                nc.tensor.matmul(
                    po[:, :nsz],
                    lhsT=v_s[:, mt, ds(h * 65, 65)],
                    rhs=exp_t[:, mt, ds(no, nsz)],
                    start=(mt == 0),
                    stop=(mt == NT - 1),
                )
            # softmax denominator, off the PE critical path: one broadcast-DMA
            # of the S row (psum part 64) to 64 partitions, then a full-width
            # fast reciprocal and the normalize multiply on DVE.
            nci = no // 512
            s_row = small.tile([65, 512], f32, tag="s_row")
            nc.scalar.copy(out=s_row[64:65, :nsz], in_=po[64:65, :nsz])
            recipB = small.tile([64, 512], f32, tag="recipB")
            r_dram = t["r_scratch"][h, nci, :, :nsz]  # [1, nsz] DRAM
            nc.sync.dma_start(out=r_dram, in_=s_row[64:65, :nsz])
            nc.sync.dma_start(
                out=recipB[:, :nsz], in_=r_dram.to_broadcast((64, nsz))
            )
            nc.vector.reciprocal_approx_fast(
                out=recipB[:, :nsz], in_=recipB[:, :nsz]
            )
            if h % 2 == 0:
                nc.vector.tensor_mul(
                    out=ao_s[0:64, h // 2, ds(no, nsz)],
                    in0=po[:64, :nsz],
                    in1=recipB[:, :nsz],
                )
            else:
                # odd heads land on partitions 64-127 of the pair tile; DVE
                # can't shift partitions, so normalize into a temp and DMA.
                ao_tmp = small.tile([64, 512], bf, tag="ao_tmp")
                nc.vector.tensor_mul(
                    out=ao_tmp[:, :nsz], in0=po[:64, :nsz], in1=recipB[:, :nsz]
                )
                nc.sync.dma_start(
                    out=ao_s[64:128, h // 2, ds(no, nsz)], in_=ao_tmp[:, :nsz]
                )

    prev = None
    head_order = list(range(H_))
    if H_ >= 2:
        head_order[-2], head_order[-1] = head_order[-1], head_order[-2]
    for h in head_order:
        kh_t = kh_pool.tile([128, CT, N_], qdt, tag="kh")
        exp_t = exp_pool.tile([128, NT, N_], bf, tag="exp")
        emit_scores(h, kh_t, exp_t)
        if prev is not None:
            emit_av(prev[0], prev[1])
        prev = (h, exp_t)

    # ---- output projection + bproj ----
    # Pairs 0..GP-2 (heads 0..H-3) are final once av(H-3)'s normalize lands,
    # so their proj matmuls are emitted BEFORE the last head's AV to fill the
    # kernel-tail PE gap; the last pair accumulates on top from SBUF.
    if GP > 1:
        yacc_s = singles.tile([128, NT, C_], f32, tag="yacc")
        for nt in range(NT):
            for (co, csz) in CCH:
                ps = psum_y.tile([128, 512], f32, tag="psy")
                for g in range(GP - 1):
                    nc.tensor.matmul(
                        ps[:, :csz],
                        lhsT=ao_s[:, g, ts(nt, 128)],
                        rhs=wproj_s[:, g, ds(co, csz)],
                        start=(g == 0),
                        stop=(g == GP - 2),
                    )
                nc.vector.tensor_add(
                    out=yacc_s[:, nt, ds(co, csz)],
                    in0=ps[:, :csz],
                    in1=bprojB_s[:, ds(co, csz)],
                )

    emit_av(prev[0], prev[1], fast_tail=True)

    for nt in range(NT):
        yst = ystage.tile([128, C_], f32, tag="yst")
        for (co, csz) in CCH:
            # alternate pools: scores' pool is free by now, doubling the
            # banks in flight so the DVE adds never stall the matmuls
            if (nt * len(CCH) + (co // 384)) % 2 == 0:
                ps = psum_y.tile([128, 512], f32, tag="psy")
            else:
                ps = psum.tile([128, 512], f32, tag="ps")
            nc.tensor.matmul(
                ps[:, :csz],
                lhsT=ao_s[:, GP - 1, ts(nt, 128)],
                rhs=wproj_s[:, GP - 1, ds(co, csz)],
                start=True,
                stop=True,
            )
            if GP > 1:
                nc.vector.tensor_add(
                    out=yst[:, ds(co, csz)],
                    in0=ps[:, :csz],
                    in1=yacc_s[:, nt, ds(co, csz)],
                )
            else:
                nc.vector.tensor_add(
                    out=yst[:, ds(co, csz)],
                    in0=ps[:, :csz],
                    in1=bprojB_s[:, ds(co, csz)],
                )
        nc.sync.dma_start(out=t["y"][ts(nt, 128), :], in_=yst)


def build(C_=C, N_=N, H_=H, ncores=NCORES):
    import concourse.bacc as bacc
    import concourse.mybir as mybir
    import concourse.tile as tile

    dt = mybir.dt
    nc = bacc.Bacc(
        "TRN2", target_bir_lowering=False, debug=False, num_devices=ncores
    )
    VW = H_ * 65
    t = {}
    t["xT"] = nc.dram_tensor("xT", [C_, N_], dt.bfloat16, kind="ExternalInput").ap()
    qk_dt = dt.float8e4 if FP8_QKPROJ else dt.bfloat16
    if FP8_QKPROJ:
        t["xT8"] = nc.dram_tensor(
            "xT8", [C_, N_], dt.float8e4, kind="ExternalInput"
        ).ap()
    t["wqT"] = nc.dram_tensor("wqT", [C_, C_], qk_dt, kind="ExternalInput").ap()
    t["wkT"] = nc.dram_tensor("wkT", [C_, C_], qk_dt, kind="ExternalInput").ap()
    t["wvT_aug"] = nc.dram_tensor(
        "wvT_aug", [C_, VW], dt.bfloat16, kind="ExternalInput"
    ).ap()
    t["wcbT_s"] = nc.dram_tensor(
        "wcbT_s", [C_, H_], dt.bfloat16, kind="ExternalInput"
    ).ap()
    t["wmixT"] = nc.dram_tensor(
        "wmixT", [C_, H_], dt.float32, kind="ExternalInput"
    ).ap()
    t["wproj64"] = nc.dram_tensor(
        "wproj64", [128, H_ // 2, C_], dt.bfloat16, kind="ExternalInput"
    ).ap()
    t["bvB_aug"] = nc.dram_tensor(
        "bvB_aug", [128, VW], dt.float32, kind="ExternalInput"
    ).ap()
    t["bprojB"] = nc.dram_tensor(
        "bprojB", [128, C_], dt.float32, kind="ExternalInput"
    ).ap()
    t["y"] = nc.dram_tensor("y", [N_, C_], dt.float32, kind="ExternalOutput").ap()
    t["r_scratch"] = nc.dram_tensor(
        "r_scratch", [H_, (N_ + 511) // 512, 1, 512], dt.float32, kind="Internal"
    ).ap()

    from contextlib import ExitStack

    with tile.TileContext(nc) as tc:
        with ExitStack() as ctx:
            emit(ctx, tc, t, C_, N_, H_)
    nc.compile()
    return nc


def prep_inputs(x, Wq, Wk, Wv, bv, Wmix, Wcb, Wproj, bproj, C_=C, N_=N, H_=H):
    """Host-side: build per-core input maps from full inputs."""
    VW = H_ * 65
    import ml_dtypes as _md
    F8 = _md.float8_e4m3
    if FP8_QKPROJ:
        wqT = np.ascontiguousarray(np.asarray(Wq, np.float32).T * QK_UPSCALE).astype(F8)
        wkT = np.ascontiguousarray(np.asarray(Wk, np.float32).T * QK_UPSCALE).astype(F8)
    else:
        wqT = np.ascontiguousarray(np.asarray(Wq, np.float32).T).astype(BF16)
        wkT = np.ascontiguousarray(np.asarray(Wk, np.float32).T).astype(BF16)
    wvT = np.ascontiguousarray(np.asarray(Wv, np.float32).T)  # [c, j]
    wvT_aug = np.zeros((C_, VW), np.float32)
    bvB_aug = np.zeros((128, VW), np.float32)
    bv = np.asarray(bv, np.float32)
    for h in range(H_):
        wvT_aug[:, 65 * h : 65 * h + 64] = wvT[:, 64 * h : 64 * h + 64]
        bvB_aug[:, 65 * h : 65 * h + 64] = bv[64 * h : 64 * h + 64][None, :]
        bvB_aug[:, 65 * h + 64] = 1.0
    wcbT_s = (np.asarray(Wcb, np.float32).T * SCALE).astype(BF16)
    wmixT = np.ascontiguousarray(np.asarray(Wmix, np.float32).T)
    if FP8_SCORES:
        wmixT = wmixT * MIX_UPSCALE
    wprojT = np.asarray(Wproj, np.float32).T  # [j, c]
    wproj64 = np.ascontiguousarray(
        wprojT.reshape(H_ // 2, 128, C_).transpose(1, 0, 2)
    ).astype(BF16)
    bprojB = np.broadcast_to(np.asarray(bproj, np.float32), (128, C_)).copy()

    shared = {
        "wqT": wqT,
        "wkT": wkT,
        "wvT_aug": wvT_aug.astype(BF16),
        "wcbT_s": wcbT_s,
        "wmixT": wmixT,
        "wproj64": wproj64,
        "bvB_aug": bvB_aug,
        "bprojB": bprojB,
    }
    x = np.asarray(x, np.float32)
    in_maps = []
    for b in range(x.shape[0]):
        m = dict(shared)
        xb = np.ascontiguousarray(x[b].T)
        m["xT"] = xb.astype(BF16)
        if FP8_QKPROJ:
            m["xT8"] = xb.astype(F8)
        in_maps.append(m)
    return in_maps


def kernel(x, Wq, Wk, Wv, bv, Wmix, Wcb, Wproj, bproj):
    from concourse.bass_utils import run_bass_kernel_spmd

    if "nc" not in _CACHE:
        _CACHE["nc"] = build()
    nc = _CACHE["nc"]
    in_maps = prep_inputs(x, Wq, Wk, Wv, bv, Wmix, Wcb, Wproj, bproj)
    res = run_bass_kernel_spmd(nc, in_maps, core_ids=list(range(NCORES)))
    out = np.stack([res.results[b]["y"] for b in range(len(in_maps))], axis=0)
    return out.astype(np.float32)


# revision 42
# speedup vs baseline: 1.9076x; 1.0099x over previous
"""CollaborativeAttention Trainium2 kernel.

Full inputs in, full output out. Shards batch (B=8) across 8 NeuronCores,
one batch element per core (no collectives). Matmuls are bf16 with fp32
PSUM accumulation, except the score path and the q/k input projections,
which run fp8 e4m3 with DoubleRow (2 MACs/cell/cycle); host-side upscales
(MIX_UPSCALE, QK_UPSCALE) keep fp8 operands out of the denormal range and
are divided back out inside the fused exp() scale.

Per-core dataflow (batch element b), everything transposed so the feature
dim lives on partitions and no on-device transposes are ever needed:
  stage B (from host-pretransposed xT [C,N] and weights):
    qT[j,n]  = sum_c WqT[c,j] xT8[c,n]          (fp8 DoubleRow)
    kT[j,n]  = sum_c WkT[c,j] xT8[c,n]          (fp8 DoubleRow, kept bf16)
    v[m,j']  = sum_c xT[c,m] WvT_aug[c,j'] + bvB_aug
               (j' = 12 blocks of [64 v-cols | one ones-col]; the ones
               column makes the AV matmul emit the softmax denominator)
    cbT[m,h] = sum_c xT[c,m] WcbT_s[c,h]        (SCALE prefolded)
  per head h (emission software-pipelined: scores(h) then AV(h-1)):
    khT = kT * mix[h,:]   (DVE per-partition scalar, fp8 out)
    scoresT[m,n] psum = sum_d khT[d,m]^T qT8[d,n]     (fp8 DoubleRow)
    expT[m,n] = exp(scale*scoresT + cbT[m,h])         (ScalarE, fused)
    psum_o[65,n] = sum_m v_aug[m, block_h]^T expT[m,n]; row 64 = S[n]
    normalize off the PE path: S row -> DRAM -> broadcast-DMA to 64
    partitions -> reciprocal_approx_fast -> DVE multiply into ao.
    Odd heads DMA-shift to partitions 64-127 so ao packs head PAIRS
    on 128 partitions (K=128 output projection with FWL).
  output projection, split so pairs 0..4 fill the kernel-tail PE gap
  while the last head finishes; pair 5 accumulates via SBUF (+bproj).
"""

import numpy as np
import ml_dtypes

B, N, C = 8, 1024, 768
H, Dh = 12, 64
SCALE = Dh ** -0.5
NCORES = 8
BF16 = ml_dtypes.bfloat16

# fp8 (e4m3 + DoubleRow) for the score matmuls; k*mix is pre-scaled by
# MIX_UPSCALE on the host so values clear the e4m3 denormal floor, and the
# exp() scale divides it back out.
FP8_SCORES = True
MIX_UPSCALE = 32.0
# fp8 DoubleRow for the q/k input projections; Wq/Wk are upscaled by
# QK_UPSCALE on the host (their ~0.02-scale values are denormal in e4m3),
# and the exp() scale divides the product back out.
FP8_QKPROJ = True
QK_UPSCALE = 32.0

_CACHE = {}


def _chunks(total, size):
    out = []
    off = 0
    while off < total:
        out.append((off, min(size, total - off)))
        off += size
    return out


def emit(ctx, tc, t, C_, N_, H_):
    """Emit the per-core kernel body. t: dict of dram APs."""
    import concourse.mybir as mybir
    from concourse.bass import ts, ds

    nc = tc.nc
    dt = mybir.dt
    CT = C_ // 128          # c/d tiles (contraction over features)
    NT = N_ // 128          # token tiles (n or m)
    JT = C_ // 128          # output-feature tiles for q/k
    VW = H_ * 65            # augmented v width
    NCH = _chunks(N_, 512)  # n chunks for moving operand
    VCH = _chunks(VW, 512)
    CCH = _chunks(C_, 384)  # proj output chunks (<=512, 2 banks-friendly)

    singles = ctx.enter_context(tc.tile_pool(name="singles", bufs=1))
    kh_pool = ctx.enter_context(tc.tile_pool(name="khp", bufs=2))
    exp_pool = ctx.enter_context(tc.tile_pool(name="expp", bufs=2))
    small = ctx.enter_context(tc.tile_pool(name="small", bufs=4))
    ystage = ctx.enter_context(tc.tile_pool(name="ystage", bufs=3))
    psum = ctx.enter_context(tc.tile_pool(name="psum", bufs=3, space="PSUM"))
    psum_o = ctx.enter_context(tc.tile_pool(name="psum_o", bufs=3, space="PSUM"))
    psum_y = ctx.enter_context(tc.tile_pool(name="psum_y", bufs=2, space="PSUM"))

    bf = dt.bfloat16
    f32 = dt.float32
    f8 = dt.float8e4
    qdt = f8 if FP8_SCORES else bf
    exp_scale = SCALE / MIX_UPSCALE if FP8_SCORES else SCALE
    if FP8_QKPROJ:
        exp_scale = exp_scale / (QK_UPSCALE * QK_UPSCALE)

    # ---- persistent SBUF tensors ----
    GP = H_ // 2            # head pairs (proj contraction tiles of 128)
    wmixT_s = singles.tile([128, CT, H_], f32, tag="wmixT")
    wproj_s = singles.tile([128, GP, C_], bf, tag="wproj")
    bprojB_s = singles.tile([128, C_], f32, tag="bprojB")

    qT_s = singles.tile([128, JT, N_], qdt, tag="qT")
    kT_s = singles.tile([128, JT, N_], bf, tag="kT")
    v_s = singles.tile([128, NT, VW], bf, tag="v")
    cb_s = singles.tile([128, NT, H_], f32, tag="cb")
    ao_s = singles.tile([128, GP, N_], bf, tag="ao")

    # ---- stage B: projections (inputs scoped to a pool freed afterwards) ----
    with tc.tile_pool(name="stageb", bufs=1) as sbp:
        xT_s = sbp.tile([128, CT, N_], bf, tag="xT")
        qk_dt = f8 if FP8_QKPROJ else bf
        if FP8_QKPROJ:
            xT8_s = sbp.tile([128, CT, N_], qk_dt, tag="xT8")
        else:
            xT8_s = xT_s
        wqT_s = sbp.tile([128, CT, C_], qk_dt, tag="wqT")
        wkT_s = sbp.tile([128, CT, C_], qk_dt, tag="wkT")
        wvT_s = sbp.tile([128, CT, VW], bf, tag="wvT")
        wcbT_s = sbp.tile([128, CT, H_], bf, tag="wcbT")
        bvB_s = sbp.tile([128, VW], f32, tag="bvB")

        # per-c-tile DMAs, compute-first order, so matmul accumulation can
        # begin as soon as the first tiles land
        xT_d = t["xT"].rearrange("(t p) n -> p t n", p=128)
        wq_d = t["wqT"].rearrange("(t p) n -> p t n", p=128)
        wk_d = t["wkT"].rearrange("(t p) n -> p t n", p=128)
        wv_d = t["wvT_aug"].rearrange("(t p) n -> p t n", p=128)
        if FP8_QKPROJ:
            xT8_d = t["xT8"].rearrange("(t p) n -> p t n", p=128)
            for ct in range(CT):
                nc.scalar.dma_start(out=wkT_s[:, ct, :], in_=wk_d[:, ct, :])
                nc.sync.dma_start(out=xT8_s[:, ct, :], in_=xT8_d[:, ct, :])
            for ct in range(CT):
                nc.scalar.dma_start(out=wqT_s[:, ct, :], in_=wq_d[:, ct, :])
        else:
            for ct in range(CT):
                nc.scalar.dma_start(out=wkT_s[:, ct, :], in_=wk_d[:, ct, :])
            for ct in range(CT):
                nc.scalar.dma_start(out=wqT_s[:, ct, :], in_=wq_d[:, ct, :])
        for ct in range(CT):
            nc.sync.dma_start(out=xT_s[:, ct, :], in_=xT_d[:, ct, :])
        nc.scalar.dma_start(
            out=wcbT_s, in_=t["wcbT_s"].rearrange("(t p) n -> p t n", p=128)
        )
        nc.scalar.dma_start(
            out=wmixT_s, in_=t["wmixT"].rearrange("(t p) n -> p t n", p=128)
        )
        for ct in range(CT):
            nc.sync.dma_start(out=wvT_s[:, ct, :], in_=wv_d[:, ct, :])
        nc.sync.dma_start(out=bvB_s, in_=t["bvB_aug"])
        nc.sync.dma_start(out=wproj_s, in_=t["wproj64"])
        nc.sync.dma_start(out=bprojB_s, in_=t["bprojB"])

        # kT then qT (kT needed first for head-0 mix-scale)
        for dst, w_s in ((kT_s, wkT_s), (qT_s, wqT_s)):
            for jt in range(JT):
                for (no, nsz) in NCH:
                    ps = psum.tile([128, 512], f32, tag="ps")
                    if FP8_QKPROJ:
                        for ct in range(0, CT, 2):
                            nc.tensor.matmul(
                                ps[:, :nsz],
                                lhsT=w_s[:, ct : ct + 2, ts(jt, 128)],
                                rhs=xT8_s[:, ct : ct + 2, ds(no, nsz)],
                                start=(ct == 0),
                                stop=(ct == CT - 2),
                                perf_mode=mybir.MatmulPerfMode.DoubleRow,
                            )
                    else:
                        for ct in range(CT):
                            nc.tensor.matmul(
                                ps[:, :nsz],
                                lhsT=w_s[:, ct, ts(jt, 128)],
                                rhs=xT_s[:, ct, ds(no, nsz)],
                                start=(ct == 0),
                                stop=(ct == CT - 1),
                            )
                    nc.any.tensor_copy(out=dst[:, jt, ds(no, nsz)], in_=ps[:, :nsz])

        # content bias (needed before head-0's exp)
        for mt in range(NT):
            ps = psum.tile([128, 512], f32, tag="ps")
            for ct in range(CT):
                nc.tensor.matmul(
                    ps[:, :H_],
                    lhsT=xT_s[:, ct, ts(mt, 128)],
                    rhs=wcbT_s[:, ct, :],
                    start=(ct == 0),
                    stop=(ct == CT - 1),
                )
            nc.scalar.copy(out=cb_s[:, mt, :], in_=ps[:, :H_])

        # v (n-major, augmented with ones cols) + bias add
        for mt in range(NT):
            for (vo, vsz) in VCH:
                ps = psum.tile([128, 512], f32, tag="ps")
                for ct in range(CT):
                    nc.tensor.matmul(
                        ps[:, :vsz],
                        lhsT=xT_s[:, ct, ts(mt, 128)],
                        rhs=wvT_s[:, ct, ds(vo, vsz)],
                        start=(ct == 0),
                        stop=(ct == CT - 1),
                    )
                nc.vector.tensor_add(
                    out=v_s[:, mt, ds(vo, vsz)],
                    in0=ps[:, :vsz],
                    in1=bvB_s[:, ds(vo, vsz)],
                )

    # ---- head loop (software-pipelined emission: scores(h) then AV(h-1)) ----
    def emit_scores(h, kh_t, exp_t):
        for dt_i in range(CT):
            nc.vector.tensor_scalar_mul(
                kh_t[:, dt_i, :], kT_s[:, dt_i, :], wmixT_s[:, dt_i, h : h + 1]
            )
        for mt in range(NT):
            for (no, nsz) in NCH:
                ps = psum.tile([128, 512], f32, tag="ps")
                if FP8_SCORES:
                    for di in range(0, CT, 2):
                        nc.tensor.matmul(
                            ps[:, :nsz],
                            lhsT=kh_t[:, di : di + 2, ts(mt, 128)],
                            rhs=qT_s[:, di : di + 2, ds(no, nsz)],
                            start=(di == 0),
                            stop=(di == CT - 2),
                            perf_mode=mybir.MatmulPerfMode.DoubleRow,
                        )
                else:
                    for di in range(CT):
                        nc.tensor.matmul(
                            ps[:, :nsz],
                            lhsT=kh_t[:, di, ts(mt, 128)],
                            rhs=qT_s[:, di, ds(no, nsz)],
                            start=(di == 0),
                            stop=(di == CT - 1),
                        )
                nc.scalar.activation(
                    out=exp_t[:, mt, ds(no, nsz)],
                    in_=ps[:, :nsz],
                    func=mybir.ActivationFunctionType.Exp,
                    bias=cb_s[:, mt, h : h + 1],
                    scale=exp_scale,
                )

    def emit_av(h, exp_t, fast_tail=False):
        for (no, nsz) in NCH:
            po = psum_o.tile([65, 512], f32, tag="po")
            for mt in range(NT):
                nc.tensor.matmul(
                    po[:, :nsz],
                    lhsT=v_s[:, mt, ds(h * 65, 65)],
                    rhs=exp_t[:, mt, ds(no, nsz)],
                    start=(mt == 0),
                    stop=(mt == NT - 1),
                )
            # softmax denominator, off the PE critical path: one broadcast-DMA
            # of the S row (psum part 64) to 64 partitions, then a full-width
            # fast reciprocal and the normalize multiply on DVE.
            nci = no // 512
            s_row = small.tile([65, 512], f32, tag="s_row")
            nc.scalar.copy(out=s_row[64:65, :nsz], in_=po[64:65, :nsz])
            recipB = small.tile([64, 512], f32, tag="recipB")
            r_dram = t["r_scratch"][h, nci, :, :nsz]  # [1, nsz] DRAM
            nc.sync.dma_start(out=r_dram, in_=s_row[64:65, :nsz])
            nc.sync.dma_start(
                out=recipB[:, :nsz], in_=r_dram.to_broadcast((64, nsz))
            )
            nc.vector.reciprocal_approx_fast(
                out=recipB[:, :nsz], in_=recipB[:, :nsz]
            )
            if h % 2 == 0:
                nc.vector.tensor_mul(
                    out=ao_s[0:64, h // 2, ds(no, nsz)],
                    in0=po[:64, :nsz],
                    in1=recipB[:, :nsz],
                )
            else:
                # odd heads land on partitions 64-127 of the pair tile; DVE
                # can't shift partitions, so normalize into a temp and DMA.
                ao_tmp = small.tile([64, 512], bf, tag="ao_tmp")
                nc.vector.tensor_mul(
                    out=ao_tmp[:, :nsz], in0=po[:64, :nsz], in1=recipB[:, :nsz]
                )
                nc.sync.dma_start(
                    out=ao_s[64:128, h // 2, ds(no, nsz)], in_=ao_tmp[:, :nsz]
                )

    prev = None
    head_order = list(range(H_))
    if H_ >= 2:
        head_order[-2], head_order[-1] = head_order[-1], head_order[-2]
    for h in head_order:
        kh_t = kh_pool.tile([128, CT, N_], qdt, tag="kh")
        exp_t = exp_pool.tile([128, NT, N_], bf, tag="exp")
        emit_scores(h, kh_t, exp_t)
        if prev is not None:
            emit_av(prev[0], prev[1])
        prev = (h, exp_t)

    # ---- output projection + bproj ----
    # Pairs 0..GP-2 (heads 0..H-3) are final once av(H-3)'s normalize lands,
    # so their proj matmuls are emitted BEFORE the last head's AV to fill the
    # kernel-tail PE gap; the last pair accumulates on top from SBUF.
    if GP > 1:
        yacc_s = singles.tile([128, NT, C_], f32, tag="yacc")
        for nt in range(NT):
            for (co, csz) in CCH:
                if (nt * len(CCH) + (co // 384)) % 2 == 0:
                    ps = psum_y.tile([128, 512], f32, tag="psy")
                else:
                    ps = psum.tile([128, 512], f32, tag="ps")
                for g in range(GP - 1):
                    nc.tensor.matmul(
                        ps[:, :csz],
                        lhsT=ao_s[:, g, ts(nt, 128)],
                        rhs=wproj_s[:, g, ds(co, csz)],
                        start=(g == 0),
                        stop=(g == GP - 2),
                    )
                nc.vector.tensor_add(
                    out=yacc_s[:, nt, ds(co, csz)],
                    in0=ps[:, :csz],
                    in1=bprojB_s[:, ds(co, csz)],
                )

    emit_av(prev[0], prev[1], fast_tail=True)

    for nt in range(NT):
        yst = ystage.tile([128, C_], f32, tag="yst")
        for (co, csz) in CCH:
            # alternate pools: scores' pool is free by now, doubling the
            # banks in flight so the DVE adds never stall the matmuls
            if (nt * len(CCH) + (co // 384)) % 2 == 0:
                ps = psum_y.tile([128, 512], f32, tag="psy")
            else:
                ps = psum.tile([128, 512], f32, tag="ps")
            nc.tensor.matmul(
                ps[:, :csz],
                lhsT=ao_s[:, GP - 1, ts(nt, 128)],
                rhs=wproj_s[:, GP - 1, ds(co, csz)],
                start=True,
                stop=True,
            )
            if GP > 1:
                nc.vector.tensor_add(
                    out=yst[:, ds(co, csz)],
                    in0=ps[:, :csz],
                    in1=yacc_s[:, nt, ds(co, csz)],
                )
            else:
                nc.vector.tensor_add(
                    out=yst[:, ds(co, csz)],
                    in0=ps[:, :csz],
                    in1=bprojB_s[:, ds(co, csz)],
                )
        nc.sync.dma_start(out=t["y"][ts(nt, 128), :], in_=yst)


def build(C_=C, N_=N, H_=H, ncores=NCORES):
    import concourse.bacc as bacc
    import concourse.mybir as mybir
    import concourse.tile as tile

    dt = mybir.dt
    nc = bacc.Bacc(
        "TRN2", target_bir_lowering=False, debug=False, num_devices=ncores
    )
    VW = H_ * 65
    t = {}
    t["xT"] = nc.dram_tensor("xT", [C_, N_], dt.bfloat16, kind="ExternalInput").ap()
    qk_dt = dt.float8e4 if FP8_QKPROJ else dt.bfloat16
    if FP8_QKPROJ:
        t["xT8"] = nc.dram_tensor(
            "xT8", [C_, N_], dt.float8e4, kind="ExternalInput"
        ).ap()
    t["wqT"] = nc.dram_tensor("wqT", [C_, C_], qk_dt, kind="ExternalInput").ap()
    t["wkT"] = nc.dram_tensor("wkT", [C_, C_], qk_dt, kind="ExternalInput").ap()
    t["wvT_aug"] = nc.dram_tensor(
        "wvT_aug", [C_, VW], dt.bfloat16, kind="ExternalInput"
    ).ap()
    t["wcbT_s"] = nc.dram_tensor(
        "wcbT_s", [C_, H_], dt.bfloat16, kind="ExternalInput"
    ).ap()
    t["wmixT"] = nc.dram_tensor(
        "wmixT", [C_, H_], dt.float32, kind="ExternalInput"
    ).ap()
    t["wproj64"] = nc.dram_tensor(
        "wproj64", [128, H_ // 2, C_], dt.bfloat16, kind="ExternalInput"
    ).ap()
    t["bvB_aug"] = nc.dram_tensor(
        "bvB_aug", [128, VW], dt.float32, kind="ExternalInput"
    ).ap()
    t["bprojB"] = nc.dram_tensor(
        "bprojB", [128, C_], dt.float32, kind="ExternalInput"
    ).ap()
    t["y"] = nc.dram_tensor("y", [N_, C_], dt.float32, kind="ExternalOutput").ap()
    t["r_scratch"] = nc.dram_tensor(
        "r_scratch", [H_, (N_ + 511) // 512, 1, 512], dt.float32, kind="Internal"
    ).ap()

    from contextlib import ExitStack

    with tile.TileContext(nc) as tc:
        with ExitStack() as ctx:
            emit(ctx, tc, t, C_, N_, H_)
    nc.compile()
    return nc


def prep_inputs(x, Wq, Wk, Wv, bv, Wmix, Wcb, Wproj, bproj, C_=C, N_=N, H_=H):
    """Host-side: build per-core input maps from full inputs."""
    VW = H_ * 65
    import ml_dtypes as _md
    F8 = _md.float8_e4m3
    if FP8_QKPROJ:
        wqT = np.ascontiguousarray(np.asarray(Wq, np.float32).T * QK_UPSCALE).astype(F8)
        wkT = np.ascontiguousarray(np.asarray(Wk, np.float32).T * QK_UPSCALE).astype(F8)
    else:
        wqT = np.ascontiguousarray(np.asarray(Wq, np.float32).T).astype(BF16)
        wkT = np.ascontiguousarray(np.asarray(Wk, np.float32).T).astype(BF16)
    wvT = np.ascontiguousarray(np.asarray(Wv, np.float32).T)  # [c, j]
    wvT_aug = np.zeros((C_, VW), np.float32)
    bvB_aug = np.zeros((128, VW), np.float32)
    bv = np.asarray(bv, np.float32)
    for h in range(H_):
        wvT_aug[:, 65 * h : 65 * h + 64] = wvT[:, 64 * h : 64 * h + 64]
        bvB_aug[:, 65 * h : 65 * h + 64] = bv[64 * h : 64 * h + 64][None, :]
        bvB_aug[:, 65 * h + 64] = 1.0
    wcbT_s = (np.asarray(Wcb, np.float32).T * SCALE).astype(BF16)
    wmixT = np.ascontiguousarray(np.asarray(Wmix, np.float32).T)
    if FP8_SCORES:
        wmixT = wmixT * MIX_UPSCALE
    wprojT = np.asarray(Wproj, np.float32).T  # [j, c]
    wproj64 = np.ascontiguousarray(
        wprojT.reshape(H_ // 2, 128, C_).transpose(1, 0, 2)
    ).astype(BF16)
    bprojB = np.broadcast_to(np.asarray(bproj, np.float32), (128, C_)).copy()

    shared = {
        "wqT": wqT,
        "wkT": wkT,
        "wvT_aug": wvT_aug.astype(BF16),
        "wcbT_s": wcbT_s,
        "wmixT": wmixT,
        "wproj64": wproj64,
        "bvB_aug": bvB_aug,
        "bprojB": bprojB,
    }
    x = np.asarray(x, np.float32)
    in_maps = []
    for b in range(x.shape[0]):
        m = dict(shared)
        xb = np.ascontiguousarray(x[b].T)
        m["xT"] = xb.astype(BF16)
        if FP8_QKPROJ:
            m["xT8"] = xb.astype(F8)
        in_maps.append(m)
    return in_maps


def kernel(x, Wq, Wk, Wv, bv, Wmix, Wcb, Wproj, bproj):
    from concourse.bass_utils import run_bass_kernel_spmd

    if "nc" not in _CACHE:
        _CACHE["nc"] = build()
    nc = _CACHE["nc"]
    in_maps = prep_inputs(x, Wq, Wk, Wv, bv, Wmix, Wcb, Wproj, bproj)
    res = run_bass_kernel_spmd(nc, in_maps, core_ids=list(range(NCORES)))
    out = np.stack([res.results[b]["y"] for b in range(len(in_maps))], axis=0)
    return out.astype(np.float32)


# revision 43
# speedup vs baseline: 1.9286x; 1.0110x over previous
"""CollaborativeAttention Trainium2 kernel.

Full inputs in, full output out. Shards batch (B=8) across 8 NeuronCores,
one batch element per core (no collectives). Matmuls are bf16 with fp32
PSUM accumulation, except the score path and the q/k input projections,
which run fp8 e4m3 with DoubleRow (2 MACs/cell/cycle); host-side upscales
(MIX_UPSCALE, QK_UPSCALE) keep fp8 operands out of the denormal range and
are divided back out inside the fused exp() scale.

Per-core dataflow (batch element b), everything transposed so the feature
dim lives on partitions and no on-device transposes are ever needed:
  stage B (from host-pretransposed xT [C,N] and weights):
    qT[j,n]  = sum_c WqT[c,j] xT8[c,n]          (fp8 DoubleRow)
    kT[j,n]  = sum_c WkT[c,j] xT8[c,n]          (fp8 DoubleRow, kept bf16)
    v[m,j']  = sum_c xT[c,m] WvT_aug[c,j'] + bvB_aug
               (j' = 12 blocks of [64 v-cols | one ones-col]; the ones
               column makes the AV matmul emit the softmax denominator)
    cbT[m,h] = sum_c xT[c,m] WcbT_s[c,h]        (SCALE prefolded)
  per head h (emission software-pipelined: scores(h) then AV(h-1)):
    khT = kT * mix[h,:]   (DVE per-partition scalar, fp8 out)
    scoresT[m,n] psum = sum_d khT[d,m]^T qT8[d,n]     (fp8 DoubleRow)
    expT[m,n] = exp(scale*scoresT + cbT[m,h])         (ScalarE, fused)
    psum_o[65,n] = sum_m v_aug[m, block_h]^T expT[m,n]; row 64 = S[n]
    normalize off the PE path: S row -> DRAM -> broadcast-DMA to 64
    partitions -> reciprocal_approx_fast -> DVE multiply into ao.
    Odd heads DMA-shift to partitions 64-127 so ao packs head PAIRS
    on 128 partitions (K=128 output projection with FWL).
  output projection, split so pairs 0..4 fill the kernel-tail PE gap
  while the last head finishes; pair 5 accumulates via SBUF (+bproj).
"""

import numpy as np
import ml_dtypes

B, N, C = 8, 1024, 768
H, Dh = 12, 64
SCALE = Dh ** -0.5
NCORES = 8
BF16 = ml_dtypes.bfloat16

# fp8 (e4m3 + DoubleRow) for the score matmuls; k*mix is pre-scaled by
# MIX_UPSCALE on the host so values clear the e4m3 denormal floor, and the
# exp() scale divides it back out.
FP8_SCORES = True
MIX_UPSCALE = 32.0
# fp8 DoubleRow for the q/k input projections; Wq/Wk are upscaled by
# QK_UPSCALE on the host (their ~0.02-scale values are denormal in e4m3),
# and the exp() scale divides the product back out.
FP8_QKPROJ = True
QK_UPSCALE = 32.0

_CACHE = {}


def _chunks(total, size):
    out = []
    off = 0
    while off < total:
        out.append((off, min(size, total - off)))
        off += size
    return out


def emit(ctx, tc, t, C_, N_, H_):
    """Emit the per-core kernel body. t: dict of dram APs."""
    import concourse.mybir as mybir
    from concourse.bass import ts, ds

    nc = tc.nc
    dt = mybir.dt
    CT = C_ // 128          # c/d tiles (contraction over features)
    NT = N_ // 128          # token tiles (n or m)
    JT = C_ // 128          # output-feature tiles for q/k
    VW = H_ * 65            # augmented v width
    NCH = _chunks(N_, 512)  # n chunks for moving operand
    VCH = _chunks(VW, 512)
    CCH = _chunks(C_, 384)  # proj output chunks (<=512, 2 banks-friendly)

    singles = ctx.enter_context(tc.tile_pool(name="singles", bufs=1))
    kh_pool = ctx.enter_context(tc.tile_pool(name="khp", bufs=2))
    exp_pool = ctx.enter_context(tc.tile_pool(name="expp", bufs=2))
    small = ctx.enter_context(tc.tile_pool(name="small", bufs=4))
    ystage = ctx.enter_context(tc.tile_pool(name="ystage", bufs=3))
    psum = ctx.enter_context(tc.tile_pool(name="psum", bufs=3, space="PSUM"))
    psum_o = ctx.enter_context(tc.tile_pool(name="psum_o", bufs=3, space="PSUM"))
    psum_y = ctx.enter_context(tc.tile_pool(name="psum_y", bufs=2, space="PSUM"))

    bf = dt.bfloat16
    f32 = dt.float32
    f8 = dt.float8e4
    qdt = f8 if FP8_SCORES else bf
    exp_scale = SCALE / MIX_UPSCALE if FP8_SCORES else SCALE
    if FP8_QKPROJ:
        exp_scale = exp_scale / (QK_UPSCALE * QK_UPSCALE)

    # ---- persistent SBUF tensors ----
    GP = H_ // 2            # head pairs (proj contraction tiles of 128)
    wmixT_s = singles.tile([128, CT, H_], f32, tag="wmixT")
    wproj_s = singles.tile([128, GP, C_], bf, tag="wproj")
    bprojB_s = singles.tile([128, C_], f32, tag="bprojB")

    qT_s = singles.tile([128, JT, N_], qdt, tag="qT")
    kT_s = singles.tile([128, JT, N_], bf, tag="kT")
    v_s = singles.tile([128, NT, VW], bf, tag="v")
    cb_s = singles.tile([128, NT, H_], f32, tag="cb")
    ao_s = singles.tile([128, GP, N_], bf, tag="ao")

    # ---- stage B: projections (inputs scoped to a pool freed afterwards) ----
    with tc.tile_pool(name="stageb", bufs=1) as sbp:
        xT_s = sbp.tile([128, CT, N_], bf, tag="xT")
        qk_dt = f8 if FP8_QKPROJ else bf
        if FP8_QKPROJ:
            xT8_s = sbp.tile([128, CT, N_], qk_dt, tag="xT8")
        else:
            xT8_s = xT_s
        wqT_s = sbp.tile([128, CT, C_], qk_dt, tag="wqT")
        wkT_s = sbp.tile([128, CT, C_], qk_dt, tag="wkT")
        wvT_s = sbp.tile([128, CT, VW], bf, tag="wvT")
        wcbT_s = sbp.tile([128, CT, H_], bf, tag="wcbT")
        bvB_s = sbp.tile([128, VW], f32, tag="bvB")

        # per-c-tile DMAs, compute-first order, so matmul accumulation can
        # begin as soon as the first tiles land
        xT_d = t["xT"].rearrange("(t p) n -> p t n", p=128)
        wq_d = t["wqT"].rearrange("(t p) n -> p t n", p=128)
        wk_d = t["wkT"].rearrange("(t p) n -> p t n", p=128)
        wv_d = t["wvT_aug"].rearrange("(t p) n -> p t n", p=128)
        if FP8_QKPROJ:
            xT8_d = t["xT8"].rearrange("(t p) n -> p t n", p=128)
            for ct in range(CT):
                nc.scalar.dma_start(out=wkT_s[:, ct, :], in_=wk_d[:, ct, :])
                nc.sync.dma_start(out=xT8_s[:, ct, :], in_=xT8_d[:, ct, :])
            for ct in range(CT):
                nc.scalar.dma_start(out=wqT_s[:, ct, :], in_=wq_d[:, ct, :])
        else:
            for ct in range(CT):
                nc.scalar.dma_start(out=wkT_s[:, ct, :], in_=wk_d[:, ct, :])
            for ct in range(CT):
                nc.scalar.dma_start(out=wqT_s[:, ct, :], in_=wq_d[:, ct, :])
        for ct in range(CT):
            nc.sync.dma_start(out=xT_s[:, ct, :], in_=xT_d[:, ct, :])
        nc.scalar.dma_start(
            out=wcbT_s, in_=t["wcbT_s"].rearrange("(t p) n -> p t n", p=128)
        )
        nc.scalar.dma_start(
            out=wmixT_s, in_=t["wmixT"].rearrange("(t p) n -> p t n", p=128)
        )
        for ct in range(CT):
            nc.sync.dma_start(out=wvT_s[:, ct, :], in_=wv_d[:, ct, :])
        nc.sync.dma_start(out=bvB_s, in_=t["bvB_aug"])
        nc.sync.dma_start(out=wproj_s, in_=t["wproj64"])
        nc.sync.dma_start(out=bprojB_s, in_=t["bprojB"])

        # kT then qT (kT needed first for head-0 mix-scale)
        for dst, w_s in ((kT_s, wkT_s), (qT_s, wqT_s)):
            for jt in range(JT):
                for (no, nsz) in NCH:
                    ps = psum.tile([128, 512], f32, tag="ps")
                    if FP8_QKPROJ:
                        for ct in range(0, CT, 2):
                            nc.tensor.matmul(
                                ps[:, :nsz],
                                lhsT=w_s[:, ct : ct + 2, ts(jt, 128)],
                                rhs=xT8_s[:, ct : ct + 2, ds(no, nsz)],
                                start=(ct == 0),
                                stop=(ct == CT - 2),
                                perf_mode=mybir.MatmulPerfMode.DoubleRow,
                            )
                    else:
                        for ct in range(CT):
                            nc.tensor.matmul(
                                ps[:, :nsz],
                                lhsT=w_s[:, ct, ts(jt, 128)],
                                rhs=xT_s[:, ct, ds(no, nsz)],
                                start=(ct == 0),
                                stop=(ct == CT - 1),
                            )
                    nc.any.tensor_copy(out=dst[:, jt, ds(no, nsz)], in_=ps[:, :nsz])

        # content bias (needed before head-0's exp)
        for mt in range(NT):
            ps = psum.tile([128, 512], f32, tag="ps")
            for ct in range(CT):
                nc.tensor.matmul(
                    ps[:, :H_],
                    lhsT=xT_s[:, ct, ts(mt, 128)],
                    rhs=wcbT_s[:, ct, :],
                    start=(ct == 0),
                    stop=(ct == CT - 1),
                )
            nc.scalar.copy(out=cb_s[:, mt, :], in_=ps[:, :H_])

        # v (n-major, augmented with ones cols) + bias add
        for mt in range(NT):
            for (vo, vsz) in VCH:
                ps = psum.tile([128, 512], f32, tag="ps")
                for ct in range(CT):
                    nc.tensor.matmul(
                        ps[:, :vsz],
                        lhsT=xT_s[:, ct, ts(mt, 128)],
                        rhs=wvT_s[:, ct, ds(vo, vsz)],
                        start=(ct == 0),
                        stop=(ct == CT - 1),
                    )
                nc.vector.tensor_add(
                    out=v_s[:, mt, ds(vo, vsz)],
                    in0=ps[:, :vsz],
                    in1=bvB_s[:, ds(vo, vsz)],
                )

    # ---- head loop (software-pipelined emission: scores(h) then AV(h-1)) ----
    def emit_scores(h, kh_t, exp_t):
        for dt_i in range(CT):
            nc.vector.tensor_scalar_mul(
                kh_t[:, dt_i, :], kT_s[:, dt_i, :], wmixT_s[:, dt_i, h : h + 1]
            )
        for mt in range(NT):
            for (no, nsz) in NCH:
                ps = psum.tile([128, 512], f32, tag="ps")
                if FP8_SCORES:
                    for di in range(0, CT, 2):
                        nc.tensor.matmul(
                            ps[:, :nsz],
                            lhsT=kh_t[:, di : di + 2, ts(mt, 128)],
                            rhs=qT_s[:, di : di + 2, ds(no, nsz)],
                            start=(di == 0),
                            stop=(di == CT - 2),
                            perf_mode=mybir.MatmulPerfMode.DoubleRow,
                        )
                else:
                    for di in range(CT):
                        nc.tensor.matmul(
                            ps[:, :nsz],
                            lhsT=kh_t[:, di, ts(mt, 128)],
                            rhs=qT_s[:, di, ds(no, nsz)],
                            start=(di == 0),
                            stop=(di == CT - 1),
                        )
                nc.scalar.activation(
                    out=exp_t[:, mt, ds(no, nsz)],
                    in_=ps[:, :nsz],
                    func=mybir.ActivationFunctionType.Exp,
                    bias=cb_s[:, mt, h : h + 1],
                    scale=exp_scale,
                )

    def emit_av(h, exp_t, fast_tail=False):
        for (no, nsz) in NCH:
            po = psum_o.tile([65, 512], f32, tag="po")
            for mt in range(NT):
                nc.tensor.matmul(
                    po[:, :nsz],
                    lhsT=v_s[:, mt, ds(h * 65, 65)],
                    rhs=exp_t[:, mt, ds(no, nsz)],
                    start=(mt == 0),
                    stop=(mt == NT - 1),
                )
            # softmax denominator, off the PE critical path: one broadcast-DMA
            # of the S row (psum part 64) to 64 partitions, then a full-width
            # fast reciprocal and the normalize multiply on DVE.
            nci = no // 512
            s_row = small.tile([65, 512], f32, tag="s_row")
            nc.scalar.copy(out=s_row[64:65, :nsz], in_=po[64:65, :nsz])
            recipB = small.tile([64, 512], f32, tag="recipB")
            r_dram = t["r_scratch"][h, nci, :, :nsz]  # [1, nsz] DRAM
            nc.sync.dma_start(out=r_dram, in_=s_row[64:65, :nsz])
            nc.sync.dma_start(
                out=recipB[:, :nsz], in_=r_dram.to_broadcast((64, nsz))
            )
            nc.vector.reciprocal_approx_fast(
                out=recipB[:, :nsz], in_=recipB[:, :nsz]
            )
            if h % 2 == 0:
                nc.vector.tensor_mul(
                    out=ao_s[0:64, h // 2, ds(no, nsz)],
                    in0=po[:64, :nsz],
                    in1=recipB[:, :nsz],
                )
            else:
                # odd heads land on partitions 64-127 of the pair tile; DVE
                # can't shift partitions, so normalize into a temp and DMA.
                ao_tmp = small.tile([64, 512], bf, tag="ao_tmp")
                nc.vector.tensor_mul(
                    out=ao_tmp[:, :nsz], in0=po[:64, :nsz], in1=recipB[:, :nsz]
                )
                nc.sync.dma_start(
                    out=ao_s[64:128, h // 2, ds(no, nsz)], in_=ao_tmp[:, :nsz]
                )

    prev = None
    head_order = list(range(H_))
    if H_ >= 2:
        head_order[-2], head_order[-1] = head_order[-1], head_order[-2]
    for h in head_order:
        kh_t = kh_pool.tile([128, CT, N_], qdt, tag="kh")
        exp_t = exp_pool.tile([128, NT, N_], bf, tag="exp")
        emit_scores(h, kh_t, exp_t)
        if prev is not None:
            emit_av(prev[0], prev[1])
        prev = (h, exp_t)

    # ---- output projection + bproj ----
    # Pairs 0..GP-2 (heads 0..H-3) are final once av(H-3)'s normalize lands,
    # so their proj matmuls are emitted BEFORE the last head's AV to fill the
    # kernel-tail PE gap; the last pair accumulates on top from SBUF.
    if GP > 1:
        yacc_s = singles.tile([128, NT, C_], f32, tag="yacc")
        for nt in range(NT):
            for (co, csz) in CCH:
                ps = psum_y.tile([128, 512], f32, tag="psy")
                for g in range(GP - 1):
                    nc.tensor.matmul(
                        ps[:, :csz],
                        lhsT=ao_s[:, g, ts(nt, 128)],
                        rhs=wproj_s[:, g, ds(co, csz)],
                        start=(g == 0),
                        stop=(g == GP - 2),
                    )
                nc.vector.tensor_add(
                    out=yacc_s[:, nt, ds(co, csz)],
                    in0=ps[:, :csz],
                    in1=bprojB_s[:, ds(co, csz)],
                )

    emit_av(prev[0], prev[1], fast_tail=True)

    for nt in range(NT):
        yst = ystage.tile([128, C_], f32, tag="yst")
        for (co, csz) in CCH:
            # alternate pools: scores' pool is free by now, doubling the
            # banks in flight so the DVE adds never stall the matmuls
            if (nt * len(CCH) + (co // 384)) % 2 == 0:
                ps = psum_y.tile([128, 512], f32, tag="psy")
            else:
                ps = psum.tile([128, 512], f32, tag="ps")
            nc.tensor.matmul(
                ps[:, :csz],
                lhsT=ao_s[:, GP - 1, ts(nt, 128)],
                rhs=wproj_s[:, GP - 1, ds(co, csz)],
                start=True,
                stop=True,
            )
            if GP > 1:
                nc.vector.tensor_add(
                    out=yst[:, ds(co, csz)],
                    in0=ps[:, :csz],
                    in1=yacc_s[:, nt, ds(co, csz)],
                )
            else:
                nc.vector.tensor_add(
                    out=yst[:, ds(co, csz)],
                    in0=ps[:, :csz],
                    in1=bprojB_s[:, ds(co, csz)],
                )
        nc.sync.dma_start(out=t["y"][ts(nt, 128), :], in_=yst)


def build(C_=C, N_=N, H_=H, ncores=NCORES):
    import concourse.bacc as bacc
    import concourse.mybir as mybir
    import concourse.tile as tile

    dt = mybir.dt
    nc = bacc.Bacc(
        "TRN2", target_bir_lowering=False, debug=False, num_devices=ncores
    )
    VW = H_ * 65
    t = {}
    t["xT"] = nc.dram_tensor("xT", [C_, N_], dt.bfloat16, kind="ExternalInput").ap()
    qk_dt = dt.float8e4 if FP8_QKPROJ else dt.bfloat16
    if FP8_QKPROJ:
        t["xT8"] = nc.dram_tensor(
            "xT8", [C_, N_], dt.float8e4, kind="ExternalInput"
        ).ap()
    t["wqT"] = nc.dram_tensor("wqT", [C_, C_], qk_dt, kind="ExternalInput").ap()
    t["wkT"] = nc.dram_tensor("wkT", [C_, C_], qk_dt, kind="ExternalInput").ap()
    t["wvT_aug"] = nc.dram_tensor(
        "wvT_aug", [C_, VW], dt.bfloat16, kind="ExternalInput"
    ).ap()
    t["wcbT_s"] = nc.dram_tensor(
        "wcbT_s", [C_, H_], dt.bfloat16, kind="ExternalInput"
    ).ap()
    t["wmixT"] = nc.dram_tensor(
        "wmixT", [C_, H_], dt.float32, kind="ExternalInput"
    ).ap()
    t["wproj64"] = nc.dram_tensor(
        "wproj64", [128, H_ // 2, C_], dt.bfloat16, kind="ExternalInput"
    ).ap()
    t["bvB_aug"] = nc.dram_tensor(
        "bvB_aug", [128, VW], dt.float32, kind="ExternalInput"
    ).ap()
    t["bprojB"] = nc.dram_tensor(
        "bprojB", [128, C_], dt.float32, kind="ExternalInput"
    ).ap()
    t["y"] = nc.dram_tensor("y", [N_, C_], dt.float32, kind="ExternalOutput").ap()
    t["r_scratch"] = nc.dram_tensor(
        "r_scratch", [H_, (N_ + 511) // 512, 1, 512], dt.float32, kind="Internal"
    ).ap()

    from contextlib import ExitStack

    with tile.TileContext(nc) as tc:
        with ExitStack() as ctx:
            emit(ctx, tc, t, C_, N_, H_)
    nc.compile()
    return nc


def prep_inputs(x, Wq, Wk, Wv, bv, Wmix, Wcb, Wproj, bproj, C_=C, N_=N, H_=H):
    """Host-side: build per-core input maps from full inputs."""
    VW = H_ * 65
    import ml_dtypes as _md
    F8 = _md.float8_e4m3
    if FP8_QKPROJ:
        wqT = np.ascontiguousarray(np.asarray(Wq, np.float32).T * QK_UPSCALE).astype(F8)
        wkT = np.ascontiguousarray(np.asarray(Wk, np.float32).T * QK_UPSCALE).astype(F8)
    else:
        wqT = np.ascontiguousarray(np.asarray(Wq, np.float32).T).astype(BF16)
        wkT = np.ascontiguousarray(np.asarray(Wk, np.float32).T).astype(BF16)
    wvT = np.ascontiguousarray(np.asarray(Wv, np.float32).T)  # [c, j]
    wvT_aug = np.zeros((C_, VW), np.float32)
    bvB_aug = np.zeros((128, VW), np.float32)
    bv = np.asarray(bv, np.float32)
    for h in range(H_):
        wvT_aug[:, 65 * h : 65 * h + 64] = wvT[:, 64 * h : 64 * h + 64]
        bvB_aug[:, 65 * h : 65 * h + 64] = bv[64 * h : 64 * h + 64][None, :]
        bvB_aug[:, 65 * h + 64] = 1.0
    wcbT_s = (np.asarray(Wcb, np.float32).T * SCALE).astype(BF16)
    wmixT = np.ascontiguousarray(np.asarray(Wmix, np.float32).T)
    if FP8_SCORES:
        wmixT = wmixT * MIX_UPSCALE
    wprojT = np.asarray(Wproj, np.float32).T  # [j, c]
    wproj64 = np.ascontiguousarray(
        wprojT.reshape(H_ // 2, 128, C_).transpose(1, 0, 2)
    ).astype(BF16)
    bprojB = np.broadcast_to(np.asarray(bproj, np.float32), (128, C_)).copy()

    shared = {
        "wqT": wqT,
        "wkT": wkT,
        "wvT_aug": wvT_aug.astype(BF16),
        "wcbT_s": wcbT_s,
        "wmixT": wmixT,
        "wproj64": wproj64,
        "bvB_aug": bvB_aug,
        "bprojB": bprojB,
    }
    x = np.asarray(x, np.float32)
    in_maps = []
    for b in range(x.shape[0]):
        m = dict(shared)
        xb = np.ascontiguousarray(x[b].T)
        m["xT"] = xb.astype(BF16)
        if FP8_QKPROJ:
            m["xT8"] = xb.astype(F8)
        in_maps.append(m)
    return in_maps


def kernel(x, Wq, Wk, Wv, bv, Wmix, Wcb, Wproj, bproj):
    from concourse.bass_utils import run_bass_kernel_spmd

    if "nc" not in _CACHE:
        _CACHE["nc"] = build()
    nc = _CACHE["nc"]
    in_maps = prep_inputs(x, Wq, Wk, Wv, bv, Wmix, Wcb, Wproj, bproj)
    res = run_bass_kernel_spmd(nc, in_maps, core_ids=list(range(NCORES)))
    out = np.stack([res.results[b]["y"] for b in range(len(in_maps))], axis=0)
    return out.astype(np.float32)


# revision 44
# speedup vs baseline: 1.9538x; 1.0131x over previous
"""CollaborativeAttention Trainium2 kernel.

Full inputs in, full output out. Shards batch (B=8) across 8 NeuronCores,
one batch element per core (no collectives). Matmuls are bf16 with fp32
PSUM accumulation, except the score path and the q/k input projections,
which run fp8 e4m3 with DoubleRow (2 MACs/cell/cycle); host-side upscales
(MIX_UPSCALE, QK_UPSCALE) keep fp8 operands out of the denormal range and
are divided back out inside the fused exp() scale.

Per-core dataflow (batch element b), everything transposed so the feature
dim lives on partitions and no on-device transposes are ever needed:
  stage B (from host-pretransposed xT [C,N] and weights):
    qT[j,n]  = sum_c WqT[c,j] xT8[c,n]          (fp8 DoubleRow)
    kT[j,n]  = sum_c WkT[c,j] xT8[c,n]          (fp8 DoubleRow, kept bf16)
    v[m,j']  = sum_c xT[c,m] WvT_aug[c,j'] + bvB_aug
               (j' = 12 blocks of [64 v-cols | one ones-col]; the ones
               column makes the AV matmul emit the softmax denominator)
    cbT[m,h] = sum_c xT[c,m] WcbT_s[c,h]        (SCALE prefolded)
  per head h (emission software-pipelined: scores(h) then AV(h-1)):
    khT = kT * mix[h,:]   (DVE per-partition scalar, fp8 out)
    scoresT[m,n] psum = sum_d khT[d,m]^T qT8[d,n]     (fp8 DoubleRow)
    expT[m,n] = exp(scale*scoresT + cbT[m,h])         (ScalarE, fused)
    psum_o[65,n] = sum_m v_aug[m, block_h]^T expT[m,n]; row 64 = S[n]
    normalize off the PE path: S row -> DRAM -> broadcast-DMA to 64
    partitions -> reciprocal_approx_fast -> DVE multiply into ao.
    Odd heads DMA-shift to partitions 64-127 so ao packs head PAIRS
    on 128 partitions (K=128 output projection with FWL).
  output projection, split so pairs 0..4 fill the kernel-tail PE gap
  while the last head finishes; pair 5 accumulates via SBUF (+bproj).
"""

import numpy as np
import ml_dtypes

B, N, C = 8, 1024, 768
H, Dh = 12, 64
SCALE = Dh ** -0.5
NCORES = 8
BF16 = ml_dtypes.bfloat16

# fp8 (e4m3 + DoubleRow) for the score matmuls; k*mix is pre-scaled by
# MIX_UPSCALE on the host so values clear the e4m3 denormal floor, and the
# exp() scale divides it back out.
FP8_SCORES = True
MIX_UPSCALE = 32.0
# fp8 DoubleRow for the q/k input projections; Wq/Wk are upscaled by
# QK_UPSCALE on the host (their ~0.02-scale values are denormal in e4m3),
# and the exp() scale divides the product back out.
FP8_QKPROJ = True
QK_UPSCALE = 32.0

_CACHE = {}


def _chunks(total, size):
    out = []
    off = 0
    while off < total:
        out.append((off, min(size, total - off)))
        off += size
    return out


def emit(ctx, tc, t, C_, N_, H_):
    """Emit the per-core kernel body. t: dict of dram APs."""
    import concourse.mybir as mybir
    from concourse.bass import ts, ds

    nc = tc.nc
    dt = mybir.dt
    CT = C_ // 128          # c/d tiles (contraction over features)
    NT = N_ // 128          # token tiles (n or m)
    JT = C_ // 128          # output-feature tiles for q/k
    VW = H_ * 65            # augmented v width
    NCH = _chunks(N_, 512)  # n chunks for moving operand
    VCH = _chunks(VW, 512)
    CCH = _chunks(C_, 384)  # proj output chunks (<=512, 2 banks-friendly)

    singles = ctx.enter_context(tc.tile_pool(name="singles", bufs=1))
    kh_pool = ctx.enter_context(tc.tile_pool(name="khp", bufs=2))
    exp_pool = ctx.enter_context(tc.tile_pool(name="expp", bufs=2))
    small = ctx.enter_context(tc.tile_pool(name="small", bufs=4))
    ystage = ctx.enter_context(tc.tile_pool(name="ystage", bufs=3))
    psum = ctx.enter_context(tc.tile_pool(name="psum", bufs=3, space="PSUM"))
    psum_o = ctx.enter_context(tc.tile_pool(name="psum_o", bufs=3, space="PSUM"))
    psum_y = ctx.enter_context(tc.tile_pool(name="psum_y", bufs=2, space="PSUM"))

    bf = dt.bfloat16
    f32 = dt.float32
    f8 = dt.float8e4
    qdt = f8 if FP8_SCORES else bf
    exp_scale = SCALE / MIX_UPSCALE if FP8_SCORES else SCALE
    if FP8_QKPROJ:
        exp_scale = exp_scale / (QK_UPSCALE * QK_UPSCALE)

    # ---- persistent SBUF tensors ----
    GP = H_ // 2            # head pairs (proj contraction tiles of 128)
    wmixT_s = singles.tile([128, CT, H_], f32, tag="wmixT")
    wproj_s = singles.tile([128, GP, C_], bf, tag="wproj")
    bprojB_s = singles.tile([128, C_], f32, tag="bprojB")

    qT_s = singles.tile([128, JT, N_], qdt, tag="qT")
    kT_s = singles.tile([128, JT, N_], bf, tag="kT")
    v_s = singles.tile([128, NT, VW], bf, tag="v")
    cb_s = singles.tile([128, NT, H_], f32, tag="cb")
    ao_s = singles.tile([128, GP, N_], bf, tag="ao")

    # ---- stage B: projections (inputs scoped to a pool freed afterwards) ----
    with tc.tile_pool(name="stageb", bufs=1) as sbp:
        xT_s = sbp.tile([128, CT, N_], bf, tag="xT")
        qk_dt = f8 if FP8_QKPROJ else bf
        if FP8_QKPROJ:
            xT8_s = sbp.tile([128, CT, N_], qk_dt, tag="xT8")
        else:
            xT8_s = xT_s
        wqT_s = sbp.tile([128, CT, C_], qk_dt, tag="wqT")
        wkT_s = sbp.tile([128, CT, C_], qk_dt, tag="wkT")
        wvT_s = sbp.tile([128, CT, VW], bf, tag="wvT")
        wcbT_s = sbp.tile([128, CT, H_], bf, tag="wcbT")
        bvB_s = sbp.tile([128, VW], f32, tag="bvB")

        # per-c-tile DMAs, compute-first order, so matmul accumulation can
        # begin as soon as the first tiles land
        xT_d = t["xT"].rearrange("(t p) n -> p t n", p=128)
        wq_d = t["wqT"].rearrange("(t p) n -> p t n", p=128)
        wk_d = t["wkT"].rearrange("(t p) n -> p t n", p=128)
        wv_d = t["wvT_aug"].rearrange("(t p) n -> p t n", p=128)
        if FP8_QKPROJ:
            xT8_d = t["xT8"].rearrange("(t p) n -> p t n", p=128)
            for ct in range(CT):
                nc.scalar.dma_start(out=wkT_s[:, ct, :], in_=wk_d[:, ct, :])
                nc.sync.dma_start(out=xT8_s[:, ct, :], in_=xT8_d[:, ct, :])
            for ct in range(CT):
                nc.scalar.dma_start(out=wqT_s[:, ct, :], in_=wq_d[:, ct, :])
        else:
            for ct in range(CT):
                nc.scalar.dma_start(out=wkT_s[:, ct, :], in_=wk_d[:, ct, :])
            for ct in range(CT):
                nc.scalar.dma_start(out=wqT_s[:, ct, :], in_=wq_d[:, ct, :])
        for ct in range(CT):
            nc.sync.dma_start(out=xT_s[:, ct, :], in_=xT_d[:, ct, :])
        nc.scalar.dma_start(
            out=wcbT_s, in_=t["wcbT_s"].rearrange("(t p) n -> p t n", p=128)
        )
        nc.scalar.dma_start(
            out=wmixT_s, in_=t["wmixT"].rearrange("(t p) n -> p t n", p=128)
        )
        for ct in range(CT):
            nc.sync.dma_start(out=wvT_s[:, ct, :], in_=wv_d[:, ct, :])
        nc.sync.dma_start(out=bvB_s, in_=t["bvB_aug"])
        nc.sync.dma_start(out=wproj_s, in_=t["wproj64"])
        nc.sync.dma_start(out=bprojB_s, in_=t["bprojB"])

        # kT then qT (kT needed first for head-0 mix-scale)
        for dst, w_s in ((kT_s, wkT_s), (qT_s, wqT_s)):
            for jt in range(JT):
                for (no, nsz) in NCH:
                    ps = psum.tile([128, 512], f32, tag="ps")
                    if FP8_QKPROJ:
                        for ct in range(0, CT, 2):
                            nc.tensor.matmul(
                                ps[:, :nsz],
                                lhsT=w_s[:, ct : ct + 2, ts(jt, 128)],
                                rhs=xT8_s[:, ct : ct + 2, ds(no, nsz)],
                                start=(ct == 0),
                                stop=(ct == CT - 2),
                                perf_mode=mybir.MatmulPerfMode.DoubleRow,
                            )
                    else:
                        for ct in range(CT):
                            nc.tensor.matmul(
                                ps[:, :nsz],
                                lhsT=w_s[:, ct, ts(jt, 128)],
                                rhs=xT_s[:, ct, ds(no, nsz)],
                                start=(ct == 0),
                                stop=(ct == CT - 1),
                            )
                    nc.any.tensor_copy(out=dst[:, jt, ds(no, nsz)], in_=ps[:, :nsz])

        # content bias (needed before head-0's exp)
        for mt in range(NT):
            ps = psum.tile([128, 512], f32, tag="ps")
            for ct in range(CT):
                nc.tensor.matmul(
                    ps[:, :H_],
                    lhsT=xT_s[:, ct, ts(mt, 128)],
                    rhs=wcbT_s[:, ct, :],
                    start=(ct == 0),
                    stop=(ct == CT - 1),
                )
            nc.scalar.copy(out=cb_s[:, mt, :], in_=ps[:, :H_])

        # v (n-major, augmented with ones cols) + bias add
        for mt in range(NT):
            for (vo, vsz) in VCH:
                ps = psum.tile([128, 512], f32, tag="ps")
                for ct in range(CT):
                    nc.tensor.matmul(
                        ps[:, :vsz],
                        lhsT=xT_s[:, ct, ts(mt, 128)],
                        rhs=wvT_s[:, ct, ds(vo, vsz)],
                        start=(ct == 0),
                        stop=(ct == CT - 1),
                    )
                nc.vector.tensor_add(
                    out=v_s[:, mt, ds(vo, vsz)],
                    in0=ps[:, :vsz],
                    in1=bvB_s[:, ds(vo, vsz)],
                )

    # ---- head loop (software-pipelined emission: scores(h) then AV(h-1)) ----
    def emit_scores(h, kh_t, exp_t):
        for dt_i in range(CT):
            nc.vector.tensor_scalar_mul(
                kh_t[:, dt_i, :], kT_s[:, dt_i, :], wmixT_s[:, dt_i, h : h + 1]
            )
        for mt in range(NT):
            for (no, nsz) in NCH:
                ps = psum.tile([128, 512], f32, tag="ps")
                if FP8_SCORES:
                    for di in range(0, CT, 2):
                        nc.tensor.matmul(
                            ps[:, :nsz],
                            lhsT=kh_t[:, di : di + 2, ts(mt, 128)],
                            rhs=qT_s[:, di : di + 2, ds(no, nsz)],
                            start=(di == 0),
                            stop=(di == CT - 2),
                            perf_mode=mybir.MatmulPerfMode.DoubleRow,
                        )
                else:
                    for di in range(CT):
                        nc.tensor.matmul(
                            ps[:, :nsz],
                            lhsT=kh_t[:, di, ts(mt, 128)],
                            rhs=qT_s[:, di, ds(no, nsz)],
                            start=(di == 0),
                            stop=(di == CT - 1),
                        )
                nc.scalar.activation(
                    out=exp_t[:, mt, ds(no, nsz)],
                    in_=ps[:, :nsz],
                    func=mybir.ActivationFunctionType.Exp,
                    bias=cb_s[:, mt, h : h + 1],
                    scale=exp_scale,
                )

    def emit_av(h, exp_t, fast_tail=False):
        for (no, nsz) in NCH:
            po = psum_o.tile([65, 512], f32, tag="po")
            for mt in range(NT):
                nc.tensor.matmul(
                    po[:, :nsz],
                    lhsT=v_s[:, mt, ds(h * 65, 65)],
                    rhs=exp_t[:, mt, ds(no, nsz)],
                    start=(mt == 0),
                    stop=(mt == NT - 1),
                )
            # softmax denominator, off the PE critical path: one broadcast-DMA
            # of the S row (psum part 64) to 64 partitions, then a full-width
            # fast reciprocal and the normalize multiply on DVE.
            nci = no // 512
            s_row = small.tile([65, 512], f32, tag="s_row")
            nc.scalar.copy(out=s_row[64:65, :nsz], in_=po[64:65, :nsz])
            recipB = small.tile([64, 512], f32, tag="recipB")
            r_dram = t["r_scratch"][h, nci, :, :nsz]  # [1, nsz] DRAM
            nc.sync.dma_start(out=r_dram, in_=s_row[64:65, :nsz])
            nc.sync.dma_start(
                out=recipB[:, :nsz], in_=r_dram.to_broadcast((64, nsz))
            )
            nc.vector.reciprocal_approx_fast(
                out=recipB[:, :nsz], in_=recipB[:, :nsz]
            )
            if h % 2 == 0:
                nc.vector.tensor_mul(
                    out=ao_s[0:64, h // 2, ds(no, nsz)],
                    in0=po[:64, :nsz],
                    in1=recipB[:, :nsz],
                )
            else:
                # odd heads land on partitions 64-127 of the pair tile; DVE
                # can't shift partitions, so normalize into a temp and DMA.
                ao_tmp = small.tile([64, 512], bf, tag="ao_tmp")
                nc.vector.tensor_mul(
                    out=ao_tmp[:, :nsz], in0=po[:64, :nsz], in1=recipB[:, :nsz]
                )
                nc.sync.dma_start(
                    out=ao_s[64:128, h // 2, ds(no, nsz)], in_=ao_tmp[:, :nsz]
                )

    prev = None
    head_order = list(range(H_))
    if H_ >= 2:
        head_order[-2], head_order[-1] = head_order[-1], head_order[-2]
    for h in head_order:
        kh_t = kh_pool.tile([128, CT, N_], qdt, tag="kh")
        exp_t = exp_pool.tile([128, NT, N_], bf, tag="exp")
        emit_scores(h, kh_t, exp_t)
        if prev is not None:
            emit_av(prev[0], prev[1])
        prev = (h, exp_t)

    # ---- output projection + bproj ----
    # Pairs 0..GP-2 (heads 0..H-3) are final once av(H-3)'s normalize lands,
    # so their proj matmuls are emitted BEFORE the last head's AV to fill the
    # kernel-tail PE gap; the last pair accumulates on top from SBUF.
    if GP > 1:
        yacc_s = singles.tile([128, NT, C_], f32, tag="yacc")

        def emit_part1(nts):
            for nt in nts:
                for (co, csz) in CCH:
                    ps = psum_y.tile([128, 512], f32, tag="psy")
                    for g in range(GP - 1):
                        nc.tensor.matmul(
                            ps[:, :csz],
                            lhsT=ao_s[:, g, ts(nt, 128)],
                            rhs=wproj_s[:, g, ds(co, csz)],
                            start=(g == 0),
                            stop=(g == GP - 2),
                        )
                    nc.vector.tensor_add(
                        out=yacc_s[:, nt, ds(co, csz)],
                        in0=ps[:, :csz],
                        in1=bprojB_s[:, ds(co, csz)],
                    )

        # first half covers the last head's exp drain, the last head's AV
        # runs in between, second half covers its normalize latency
        emit_part1(range(0, NT // 2))
        emit_av(prev[0], prev[1])
        emit_part1(range(NT // 2, NT))
    else:
        emit_av(prev[0], prev[1])

    for nt in range(NT):
        yst = ystage.tile([128, C_], f32, tag="yst")
        for (co, csz) in CCH:
            # alternate pools: scores' pool is free by now, doubling the
            # banks in flight so the DVE adds never stall the matmuls
            if (nt * len(CCH) + (co // 384)) % 2 == 0:
                ps = psum_y.tile([128, 512], f32, tag="psy")
            else:
                ps = psum.tile([128, 512], f32, tag="ps")
            nc.tensor.matmul(
                ps[:, :csz],
                lhsT=ao_s[:, GP - 1, ts(nt, 128)],
                rhs=wproj_s[:, GP - 1, ds(co, csz)],
                start=True,
                stop=True,
            )
            if GP > 1:
                nc.vector.tensor_add(
                    out=yst[:, ds(co, csz)],
                    in0=ps[:, :csz],
                    in1=yacc_s[:, nt, ds(co, csz)],
                )
            else:
                nc.vector.tensor_add(
                    out=yst[:, ds(co, csz)],
                    in0=ps[:, :csz],
                    in1=bprojB_s[:, ds(co, csz)],
                )
        nc.sync.dma_start(out=t["y"][ts(nt, 128), :], in_=yst)


def build(C_=C, N_=N, H_=H, ncores=NCORES):
    import concourse.bacc as bacc
    import concourse.mybir as mybir
    import concourse.tile as tile

    dt = mybir.dt
    nc = bacc.Bacc(
        "TRN2", target_bir_lowering=False, debug=False, num_devices=ncores
    )
    VW = H_ * 65
    t = {}
    t["xT"] = nc.dram_tensor("xT", [C_, N_], dt.bfloat16, kind="ExternalInput").ap()
    qk_dt = dt.float8e4 if FP8_QKPROJ else dt.bfloat16
    if FP8_QKPROJ:
        t["xT8"] = nc.dram_tensor(
            "xT8", [C_, N_], dt.float8e4, kind="ExternalInput"
        ).ap()
    t["wqT"] = nc.dram_tensor("wqT", [C_, C_], qk_dt, kind="ExternalInput").ap()
    t["wkT"] = nc.dram_tensor("wkT", [C_, C_], qk_dt, kind="ExternalInput").ap()
    t["wvT_aug"] = nc.dram_tensor(
        "wvT_aug", [C_, VW], dt.bfloat16, kind="ExternalInput"
    ).ap()
    t["wcbT_s"] = nc.dram_tensor(
        "wcbT_s", [C_, H_], dt.bfloat16, kind="ExternalInput"
    ).ap()
    t["wmixT"] = nc.dram_tensor(
        "wmixT", [C_, H_], dt.float32, kind="ExternalInput"
    ).ap()
    t["wproj64"] = nc.dram_tensor(
        "wproj64", [128, H_ // 2, C_], dt.bfloat16, kind="ExternalInput"
    ).ap()
    t["bvB_aug"] = nc.dram_tensor(
        "bvB_aug", [128, VW], dt.float32, kind="ExternalInput"
    ).ap()
    t["bprojB"] = nc.dram_tensor(
        "bprojB", [128, C_], dt.float32, kind="ExternalInput"
    ).ap()
    t["y"] = nc.dram_tensor("y", [N_, C_], dt.float32, kind="ExternalOutput").ap()
    t["r_scratch"] = nc.dram_tensor(
        "r_scratch", [H_, (N_ + 511) // 512, 1, 512], dt.float32, kind="Internal"
    ).ap()

    from contextlib import ExitStack

    with tile.TileContext(nc) as tc:
        with ExitStack() as ctx:
            emit(ctx, tc, t, C_, N_, H_)
    nc.compile()
    return nc


def prep_inputs(x, Wq, Wk, Wv, bv, Wmix, Wcb, Wproj, bproj, C_=C, N_=N, H_=H):
    """Host-side: build per-core input maps from full inputs."""
    VW = H_ * 65
    import ml_dtypes as _md
    F8 = _md.float8_e4m3
    if FP8_QKPROJ:
        wqT = np.ascontiguousarray(np.asarray(Wq, np.float32).T * QK_UPSCALE).astype(F8)
        wkT = np.ascontiguousarray(np.asarray(Wk, np.float32).T * QK_UPSCALE).astype(F8)
    else:
        wqT = np.ascontiguousarray(np.asarray(Wq, np.float32).T).astype(BF16)
        wkT = np.ascontiguousarray(np.asarray(Wk, np.float32).T).astype(BF16)
    wvT = np.ascontiguousarray(np.asarray(Wv, np.float32).T)  # [c, j]
    wvT_aug = np.zeros((C_, VW), np.float32)
    bvB_aug = np.zeros((128, VW), np.float32)
    bv = np.asarray(bv, np.float32)
    for h in range(H_):
        wvT_aug[:, 65 * h : 65 * h + 64] = wvT[:, 64 * h : 64 * h + 64]
        bvB_aug[:, 65 * h : 65 * h + 64] = bv[64 * h : 64 * h + 64][None, :]
        bvB_aug[:, 65 * h + 64] = 1.0
    wcbT_s = (np.asarray(Wcb, np.float32).T * SCALE).astype(BF16)
    wmixT = np.ascontiguousarray(np.asarray(Wmix, np.float32).T)
    if FP8_SCORES:
        wmixT = wmixT * MIX_UPSCALE
    wprojT = np.asarray(Wproj, np.float32).T  # [j, c]
    wproj64 = np.ascontiguousarray(
        wprojT.reshape(H_ // 2, 128, C_).transpose(1, 0, 2)
    ).astype(BF16)
    bprojB = np.broadcast_to(np.asarray(bproj, np.float32), (128, C_)).copy()

    shared = {
        "wqT": wqT,
        "wkT": wkT,
        "wvT_aug": wvT_aug.astype(BF16),
        "wcbT_s": wcbT_s,
        "wmixT": wmixT,
        "wproj64": wproj64,
        "bvB_aug": bvB_aug,
        "bprojB": bprojB,
    }
    x = np.asarray(x, np.float32)
    in_maps = []
    for b in range(x.shape[0]):
        m = dict(shared)
        xb = np.ascontiguousarray(x[b].T)
        m["xT"] = xb.astype(BF16)
        if FP8_QKPROJ:
            m["xT8"] = xb.astype(F8)
        in_maps.append(m)
    return in_maps


def kernel(x, Wq, Wk, Wv, bv, Wmix, Wcb, Wproj, bproj):
    from concourse.bass_utils import run_bass_kernel_spmd

    if "nc" not in _CACHE:
        _CACHE["nc"] = build()
    nc = _CACHE["nc"]
    in_maps = prep_inputs(x, Wq, Wk, Wv, bv, Wmix, Wcb, Wproj, bproj)
    res = run_bass_kernel_spmd(nc, in_maps, core_ids=list(range(NCORES)))
    out = np.stack([res.results[b]["y"] for b in range(len(in_maps))], axis=0)
    return out.astype(np.float32)
